# revision 1
# baseline (speedup 1.0000x reference)
"""BestBuddyLoss Trainium2 kernel (8-core data parallel).

Per image: p1 = unfold(x), p2 = unfold(gt),
q = concat(p2, unfold(down2(gt)), unfold(down4(gt)))  -> [3024, 27].
score(i,j) = d(p1_i,q_j) + d(p2_i,q_j) (squared-L2/d, clamped at 0).
With alpha=beta=1 the clamp/scale don't change the argmin:
  argmin_j score == argmax_j ( <p1_i + p2_i, q_j> - |q_j|^2 )
so each (i,j) tile is one K=28 f32r matmul with lhsT=[(p1+p2)^T; ones],
rhs=[q^T; -|q|^2].  The argmax over j is a single-pass fused custom DVE
op (running-max scan + select(Idx) + accum MAX; last tie wins).  The
final gather q[ind] uses dma_gather from a DRAM copy of q rows; the L1
loss is reduced on-chip to one scalar per core and summed on host.
"""

import sys

sys.path.insert(0, "/opt/trn_rl_repo")

import numpy as np

import concourse.bacc as bacc
import concourse.mybir as mybir
import concourse.tile as tile
from concourse.bass_utils import run_bass_kernel_spmd

# ---------------- problem constants (hardcoded) ----------------
B_FULL = 16
NCORES = 8
B_LOC = B_FULL // NCORES       # images per core
C, H, W = 3, 144, 144
G = 48                         # patch grid (144/3)
NI = G * G                     # 2304 query patches
D = 27                         # C*3*3
NQ = NI + (G // 2) ** 2 + (G // 4) ** 2  # 3024
KD = 33                        # contraction: 27 data + 4 zero + bias@32
KZ = 32                        # bias row partition (32-aligned)
JT = 504                       # j tile; 6 per row
NJT = NQ // JT                 # 6
IT = 128
NIT = NI // IT                 # 18
QCH = (NQ + 127) // 128        # 24 q column chunks
QROWS_PAD = QCH * 128          # 3072
QELEM = 64                     # q row padded to 64 f32 = 256B (dma_gather)
CUBIC_W = np.array([-0.09375, 0.59375, 0.59375, -0.09375], dtype=np.float32)

F32 = mybir.dt.float32
F32R = mybir.dt.float32r
I16 = mybir.dt.int16
ADD = mybir.AluOpType.add
SUB = mybir.AluOpType.subtract
MUL = mybir.AluOpType.mult
ABS = mybir.ActivationFunctionType.Abs

# ---------------- custom DVE op: single-pass argmax -------------------
from concourse.dve_spec import Spec, Src0, Idx, MaxNeg, select, scan, AluOp, maxx, lower
from concourse.dve_uop import DveOpSpec
import concourse.dve_ops as dve_ops
from concourse.dve_ops import DveOp


def _argmax_ref(in0, in1, c0, c1, c2):
    run = np.maximum.accumulate(in0, axis=-1)
    out = np.where(in0 >= run, np.arange(in0.shape[-1], dtype=np.float32),
                   -np.finfo(np.float32).max)
    acc = out.reshape(out.shape[0], -1).max(axis=-1, keepdims=True)
    return out.astype(np.float32), acc.astype(np.float32)


def _register_argmax_op():
    name = "ANT_ARGMAX_LAST"
    if name in dve_ops._SUB_OPCODE_FOR_NAME:
        return next(op for op in dve_ops.OPS if op.name == name)
    body = select(Src0 >= scan(AluOp.MAX, Src0), Idx, MaxNeg)
    spec = Spec(body=body, accum=maxx, reference=_argmax_ref)
    opcode = dve_ops._CUSTOM_DVE_ROW_BASE + len(dve_ops.OPS)
    shas = {v: DveOpSpec(name=name, opcode=opcode, uops=lower(spec, ver=v),
                         rd1_en=False).sha(v) for v in ("v3", "v4")}
    op = DveOp(name, spec, subdim=False, uops_sha=shas)
    dve_ops.OPS.append(op)
    dve_ops._SUB_OPCODE_FOR_NAME[name] = opcode
    dve_ops.CUSTOM_DVE_SPECS[name] = spec
    return op


ARGMAX_OP = _register_argmax_op()

# ---------------- host-side constants ---------------------------------


def _down_matrix(n, f):
    """M[h, i]: out[i] = sum_h M[h, i] * in[h]  (torch bicubic, offset t=.5)."""
    out_n = n // f
    M = np.zeros((n, out_n), dtype=np.float32)
    for i in range(out_n):
        base = f * i + (f // 2 - 1)
        for a in range(4):
            h = min(max(base + a - 1, 0), n - 1)
            M[h, i] += CUBIC_W[a]
    return M


def _perm_matrices():
    """PMT[:, m*128 + r]: one-hot at row (m*16 + r%16) -> out_m = Pm @ v."""
    P = np.zeros((128, 8 * 128), dtype=np.float32)
    for m in range(8):
        for r in range(128):
            P[m * 16 + r % 16, m * 128 + r] = 1.0
    return P


def make_consts():
    return {
        "cd2": np.ascontiguousarray(_down_matrix(H, 2)),  # [144, 72]
        "cd4": np.ascontiguousarray(_down_matrix(H, 4)),  # [144, 36]
        "idn": np.eye(128, dtype=np.float32),
        "pmt": _perm_matrices(),
        "neg1": np.full((D, 1), -1.0, dtype=np.float32),
        "ones128": np.ones((128, 1), dtype=np.float32),
    }


# ---------------- kernel construction ---------------------------------


def build_nc(stage="full", debug=False):
    nc = bacc.Bacc("TRN2", target_bir_lowering=False)

    x_d = nc.dram_tensor("x", [B_LOC, C, H, W], F32, kind="ExternalInput")
    gt_d = nc.dram_tensor("gt", [B_LOC, C, H, W], F32, kind="ExternalInput")
    cd2_d = nc.dram_tensor("cd2", [H, 72], F32, kind="ExternalInput")
    cd4_d = nc.dram_tensor("cd4", [H, 36], F32, kind="ExternalInput")
    idn_d = nc.dram_tensor("idn", [128, 128], F32, kind="ExternalInput")
    pmt_d = nc.dram_tensor("pmt", [128, 8 * 128], F32, kind="ExternalInput")
    neg1_d = nc.dram_tensor("neg1", [D, 1], F32, kind="ExternalInput")
    ones_d = nc.dram_tensor("ones128", [128, 1], F32, kind="ExternalInput")

    d2_d = nc.dram_tensor("scr_d2", [B_LOC, C, 72, 72], F32, kind="Internal")
    d4_d = nc.dram_tensor("scr_d4", [B_LOC, C, 36, 36], F32, kind="Internal")
    loss_d = nc.dram_tensor("loss", [1, 1], F32, kind="ExternalOutput")
    dbg = {}
    if debug:
        dbg["rh"] = nc.dram_tensor("dbg_rh", [B_LOC, D, NQ], F32, kind="ExternalOutput")
        dbg["lr"] = nc.dram_tensor("dbg_lr", [B_LOC, KD, NI], F32, kind="ExternalOutput")
        dbg["rr"] = nc.dram_tensor("dbg_rr", [B_LOC, KD, NQ], F32, kind="ExternalOutput")
        dbg["p1t"] = nc.dram_tensor("dbg_p1t", [B_LOC, D, NI], F32, kind="ExternalOutput")
        if stage in ("main", "tail", "tailperm", "tailgather", "full"):
            dbg["idxf"] = nc.dram_tensor("dbg_idxf", [B_LOC, 128, NIT], F32, kind="ExternalOutput")
        if stage in ("tail", "tailperm", "tailgather", "full"):
            dbg["widx"] = nc.dram_tensor("dbg_widx", [B_LOC, 128, 8 * NIT], I16, kind="ExternalOutput")
            dbg["sel"] = nc.dram_tensor("dbg_sel", [B_LOC, 32, NI], F32, kind="ExternalOutput")
            dbg["part"] = nc.dram_tensor("dbg_part", [128, B_LOC], F32, kind="ExternalOutput")

    with tile.TileContext(nc) as tc:
        with (
            tc.tile_pool(name="consts", bufs=1) as cpool,
            tc.tile_pool(name="stageA", bufs=1) as apool,
            tc.tile_pool(name="stageB", bufs=1) as bpool,
            tc.tile_pool(name="prep", bufs=1) as ppool,
            tc.tile_pool(name="persist", bufs=2) as spool,
            tc.tile_pool(name="score", bufs=2) as scpool,
            tc.tile_pool(name="small", bufs=2) as smpool,
            tc.tile_pool(name="psmain", bufs=2, space="PSUM") as psm,
            tc.tile_pool(name="pssmall", bufs=2, space="PSUM") as pss,
        ):
            cd2a = cpool.tile([128, 72], F32, tag="cd2a")
            cd2b = cpool.tile([16, 72], F32, tag="cd2b")
            cd4a = cpool.tile([128, 36], F32, tag="cd4a")
            cd4b = cpool.tile([16, 36], F32, tag="cd4b")
            idn_t = cpool.tile([128, 128], F32, tag="idn")
            pmt_t = cpool.tile([128, 8 * 128], F32, tag="pmt")
            neg1_t = cpool.tile([D, 1], F32, tag="neg1")
            ones_t = cpool.tile([128, 1], F32, tag="ones")
            nc.sync.dma_start(cd2a[:], cd2_d[0:128, :])
            nc.sync.dma_start(cd2b[:], cd2_d[128:144, :])
            nc.sync.dma_start(cd4a[:], cd4_d[0:128, :])
            nc.sync.dma_start(cd4b[:], cd4_d[128:144, :])
            nc.sync.dma_start(idn_t[:], idn_d[:])
            nc.sync.dma_start(pmt_t[:], pmt_d[:])
            nc.sync.dma_start(neg1_t[:], neg1_d[:])
            nc.sync.dma_start(ones_t[:], ones_d[:])

            junk = cpool.tile([128, NQ], F32, tag="junk")
            part = cpool.tile([128, B_LOC], F32, tag="part")

            def unfold_to(dram_plane_ap, g, dst_ap, eng=None):
                """dram [C, 3g, 3g] -> dst [27, g*g]; d=(c,r,s), f=(gi,gj).
                Runs in two gi-halves to halve stage-tile SBUF."""
                gh = g // 2
                szh = 3 * gh * g
                for hf in range(2):
                    at = apool.tile([9, 3 * (G // 2) * G], F32, tag="A",
                                    name=f"at{hf}")
                    for c in range(C):
                        src = dram_plane_ap[c].rearrange(
                            "(gi r) w -> r gi w", r=3
                        )[:, hf * gh : (hf + 1) * gh, :]
                        nc.sync.dma_start(at[3 * c : 3 * c + 3, :szh], src)
                    bt = bpool.tile([9, 3 * (G // 2) * G], F32, tag="B",
                                    name=f"bt{hf}")
                    rearr = at[:, :szh].rearrange(
                        "p (gi gj s) -> p s gi gj", gi=gh, gj=g, s=3
                    )
                    bt_v = bt[:, :szh].rearrange(
                        "p (s gi gj) -> p s gi gj", s=3, gi=gh
                    )
                    (eng or nc.gpsimd).tensor_copy(bt_v, rearr)
                    nc.sync.dma_start(
                        dst_ap[:, hf * gh * g : (hf + 1) * gh * g], bt[:, :szh]
                    )

            def downsample(b, f, out_dram):
                """gt[b] --bicubic/f--> out_dram [C, H/f, H/f]."""
                n = H // f
                cda = cd2a if f == 2 else cd4a
                cdb = cd2b if f == 2 else cd4b
                ga = ppool.tile([128, C * H], F32, tag="gplane_a")
                gb = ppool.tile([16, C * H], F32, tag="gplane_b")
                gsrc = gt_d[b].rearrange("c h w -> h c w")
                nc.sync.dma_start(ga[:], gsrc[0:128])
                nc.sync.dma_start(gb[:], gsrc[128:144])
                ghp = pss.tile([128, 512], F32, tag="ps")
                nc.tensor.matmul(ghp[0:n, 0 : C * H], cda[:, 0:n], ga[:],
                                 start=True, stop=False)
                nc.tensor.matmul(ghp[0:n, 0 : C * H], cdb[:, 0:n], gb[:],
                                 start=False, stop=True)
                gh = ppool.tile([72, C * H], F32, tag="gh")
                nc.scalar.copy(gh[0:n, :], ghp[0:n, 0 : C * H])
                gh3 = gh[:].rearrange("i (c w) -> i c w", c=C)
                ghta = ppool.tile([128, C * 72], F32, tag="ghta")
                ghtb = ppool.tile([16, C * 72], F32, tag="ghtb")
                ghta3 = ghta[:].rearrange("w (c i) -> w c i", c=C)
                ghtb3 = ghtb[:].rearrange("w (c i) -> w c i", c=C)
                for c in range(C):
                    tp = pss.tile([128, 512], F32, tag="ps")
                    nc.tensor.transpose(tp[0:128, 0:n], gh3[0:n, c, 0:128],
                                        idn_t[0:n, 0:n])
                    nc.scalar.copy(ghta3[:, c, 0:n], tp[0:128, 0:n])
                    tp2 = pss.tile([128, 512], F32, tag="ps")
                    nc.tensor.transpose(tp2[0:16, 0:n],
                                        gh3[0:n, c, 128:144], idn_t[0:n, 0:n])
                    nc.scalar.copy(ghtb3[:, c, 0:n], tp2[0:16, 0:n])
                g2 = ppool.tile([72, C * 72], F32, tag="g2")
                g23 = g2[:].rearrange("i (c j) -> i c j", c=C)
                for c in range(C):
                    op = pss.tile([128, 512], F32, tag="ps")
                    nc.tensor.matmul(op[0:n, 0:n], ghta3[:, c, 0:n],
                                     cda[:, 0:n], start=True, stop=False)
                    nc.tensor.matmul(op[0:n, 0:n], ghtb3[:, c, 0:n],
                                     cdb[:, 0:n], start=False, stop=True)
                    nc.scalar.copy(g23[0:n, c, 0:n], op[0:n, 0:n])
                out_ap = out_dram.rearrange("c h w -> h c w")
                nc.sync.dma_start(out_ap, g23[0:n, :, 0:n])

            lhr = [None] * B_LOC
            rhr = [None] * B_LOC
            p1ts = [None] * B_LOC
            idxf = [None] * B_LOC

            def prep(b):
                p1t = spool.tile([D, NI], F32, tag="p1t")
                rh = ppool.tile([KD, NQ], F32, tag="rh", bufs=2)
                p1ts[b] = p1t
                fast = b == 0  # image 0 preps on otherwise-idle DVE/ACT
                e_x = nc.vector if fast else nc.gpsimd
                e_g = nc.vector if fast else nc.gpsimd
                downsample(b, 2, d2_d[b])
                downsample(b, 4, d4_d[b])
                unfold_to(gt_d[b], G, rh[0:D, 0:NI], e_g)
                unfold_to(x_d[b], G, p1t[:, :], e_x)
                unfold_to(d2_d[b], G // 2, rh[0:D, NI : NI + 576], e_x)
                unfold_to(d4_d[b], G // 4, rh[0:D, NI + 576 : NQ], e_x)

                # lhsT (f32r): rows 0..26 = (p1+p2)^T, row 27 = ones
                lr = spool.tile([KD, NI], F32R, tag="lhr")
                nc.gpsimd.memset(lr[0:KZ, :].bitcast(F32), 0.0)
                nc.gpsimd.tensor_tensor(lr[0:D, :], p1t[:, :], rh[0:D, 0:NI],
                                        op=ADD)
                nc.gpsimd.memset(lr[KZ : KZ + 1, :].bitcast(F32), 1.0)
                lhr[b] = lr

                # rhs (f32r): rows 0..26 = q^T, row 27 = -|q|^2
                rr = spool.tile([KD, NQ], F32R, tag="rhr")
                nc.gpsimd.memset(rr[0:KZ, :].bitcast(F32), 0.0)
                ce = nc.vector if fast else nc.gpsimd
                ce.tensor_copy(rr[0:D, 0:NI], rh[0:D, 0:NI])
                ce.tensor_copy(rr[0:D, NI:NQ], rh[0:D, NI:NQ])
                qsq = ppool.tile([D, NQ], F32, tag="qsq")
                if fast:
                    nc.scalar.activation(qsq[:, 0:NI], rh[0:D, 0:NI],
                                         mybir.ActivationFunctionType.Square)
                    nc.scalar.activation(qsq[:, NI:NQ], rh[0:D, NI:NQ],
                                         mybir.ActivationFunctionType.Square)
                else:
                    nc.gpsimd.tensor_tensor(qsq[:, 0:NI], rh[0:D, 0:NI],
                                            rh[0:D, 0:NI], op=MUL)
                    nc.gpsimd.tensor_tensor(qsq[:, NI:NQ], rh[0:D, NI:NQ],
                                            rh[0:D, NI:NQ], op=MUL)
                for jt in range(NJT):
                    bnp = pss.tile([128, 512], F32, tag="ps")
                    nc.tensor.matmul(bnp[0:1, 0:JT], neg1_t[:],
                                     qsq[:, jt * JT : (jt + 1) * JT])
                    nc.scalar.copy(rr[KZ : KZ + 1, jt * JT : (jt + 1) * JT],
                                   bnp[0:1, 0:JT])
                rhr[b] = rr

                idxf[b] = smpool.tile([128, NIT], F32, tag="idxf", name=f"idxf{b}")
                if debug:
                    nc.sync.dma_start(dbg["rh"][b], rh[0:D, :])
                    nc.sync.dma_start(dbg["lr"][b], lr[:, :].bitcast(F32))
                    nc.sync.dma_start(dbg["rr"][b], rr[:, :].bitcast(F32))
                    nc.sync.dma_start(dbg["p1t"][b], p1t[:, :])

            def main(b):
                lr, rr = lhr[b], rhr[b]
                for t in range(NIT):
                    sc = scpool.tile([128, NQ], F32, tag="sc")
                    for third in range(NJT // 3):
                        ps = psm.tile([128, 3, 512], F32, tag="psmain")
                        for k in range(3):
                            jt = third * 3 + k
                            nc.tensor.matmul(
                                ps[:, k, 0:JT],
                                lr[:, t * IT : (t + 1) * IT],
                                rr[:, jt * JT : (jt + 1) * JT],
                            )
                        nc.scalar.copy(
                            sc[:, third * 3 * JT : (third + 1) * 3 * JT],
                            ps[:, :, 0:JT],
                        )
                    nc.vector._custom_dve(
                        ARGMAX_OP, out=junk[:], in0=sc[:],
                        accum_out=idxf[b][:, t : t + 1],
                    )
                if debug:
                    nc.sync.dma_start(dbg["idxf"][b], idxf[b][:, :])

            def tail(b):
                # permute idxf into dma_gather's wrapped index layout
                wp = pss.tile([128, 512], F32, tag="ps")
                wp3 = wp[0:128, 0 : 8 * NIT].rearrange("p (m t) -> p m t", m=8)
                for m in range(8):
                    nc.tensor.matmul(
                        wp3[:, m, :], pmt_t[:, m * 128 : (m + 1) * 128],
                        idxf[b][:, :],
                    )
                widx = smpool.tile([128, 8 * NIT], I16, tag="widx")
                w3 = widx[:].rearrange("p (t m) -> p t m", t=NIT)
                nc.vector.tensor_copy(w3[:, :, :],
                                      wp3[:, :, :].rearrange("p m t -> p t m"))
                if debug and "widx" in dbg:
                    nc.sync.dma_start(dbg["widx"][b], widx[:, :])
                if stage == "tailperm":
                    return

                sel = smpool.tile([32, NI], F32, tag="sel", bufs=1)
                nc.gpsimd.ap_gather(
                    sel[:, :].rearrange("p (n d) -> p n d", d=1),
                    rhr[b][0:32, :].bitcast(F32).rearrange(
                        "p (n d) -> p n d", d=1),
                    widx[0:32, :],
                    channels=32, num_elems=NQ, d=1, num_idxs=NI,
                )
                if debug and "sel" in dbg:
                    nc.sync.dma_start(dbg["sel"][b], sel[:, :])
                if stage == "tailgather":
                    return
                df = smpool.tile([D, NI], F32, tag="df", bufs=1)
                nc.vector.tensor_tensor(df[:, :], sel[0:D, :],
                                        p1ts[b][:, :], op=SUB)
                nc.scalar.activation(junk[0:D, 0:NI], df[:, :], ABS,
                                     accum_out=part[0:D, b : b + 1])

            for b in range(B_LOC):
                prep(b)
            if stage in ("main", "tail", "tailperm", "tailgather", "full"):
                for b in range(B_LOC):
                    main(b)
                    if stage in ("tail", "tailperm", "tailgather", "full"):
                        tail(b)

            lt = smpool.tile([1, 1], F32, tag="loss")
            if stage == "full" or stage == "tail":
                # [128, B] --ones matmul--> [1, B] --abs accum--> [1,1]
                fin = pss.tile([128, 512], F32, tag="ps")
                nc.tensor.matmul(fin[0:1, 0:B_LOC], ones_t[0:D, :],
                                 part[0:D, :])
                ltj = smpool.tile([1, B_LOC], F32, tag="ltj")
                nc.scalar.activation(ltj[:], fin[0:1, 0:B_LOC], ABS,
                                     accum_out=lt[:])
                if debug and "part" in dbg:
                    nc.sync.dma_start(dbg["part"][:, :], part[:, :])
            else:
                nc.vector.memset(lt[:], 0.0)
            nc.sync.dma_start(loss_d[:], lt[:])

    nc.compile()
    return nc


_NC_CACHE = None


def _get_nc():
    global _NC_CACHE
    if _NC_CACHE is None:
        _NC_CACHE = build_nc()
    return _NC_CACHE


def kernel(x: np.ndarray, gt: np.ndarray, _trace=False):
    x = np.ascontiguousarray(np.asarray(x, dtype=np.float32))
    gt = np.ascontiguousarray(np.asarray(gt, dtype=np.float32))
    consts = make_consts()
    nc = _get_nc()
    in_maps = []
    for c in range(NCORES):
        m = {"x": x[c * B_LOC : (c + 1) * B_LOC],
             "gt": gt[c * B_LOC : (c + 1) * B_LOC]}
        m.update(consts)
        in_maps.append(m)
    res = run_bass_kernel_spmd(
        nc, in_maps, core_ids=list(range(NCORES)), trace=_trace,
        trace_cores=[0] if _trace else None,
    )
    total = sum(float(r["loss"][0, 0]) for r in res.results)
    out = np.asarray(np.float32(total / (B_FULL * NI * D)))
    if _trace:
        return out, res
    return out


if __name__ == "__main__":
    xs = np.load("/root/problem/work/x.npy")
    gts = np.load("/root/problem/work/gt.npy")
    expected = float(np.load("/root/problem/work/expected.npy"))
    got = float(kernel(xs, gts))
    rel = abs(got - expected) / abs(expected)
    print(f"expected {expected:.8f}  got {got:.8f}  relerr {rel:.3e}")



# revision 15
# speedup vs baseline: 1.1241x; 1.1241x over previous
"""BestBuddyLoss Trainium2 kernel (8-core data parallel).

Per image: p1 = unfold(x), p2 = unfold(gt),
q = concat(p2, unfold(down2(gt)), unfold(down4(gt)))  -> [3024, 27].
score(i,j) = d(p1_i,q_j) + d(p2_i,q_j) (squared-L2/d, clamped at 0).
With alpha=beta=1 the clamp/scale don't change the argmin:
  argmin_j score == argmax_j ( <p1_i + p2_i, q_j> - |q_j|^2 )
so each (i,j) tile is one K=33 f32r matmul with lhsT=[(p1+p2)^T; ones],
rhs=[q^T; -|q|^2].  The argmax over j is a single-pass fused custom DVE
op (running-max scan + select(Idx) + accum MAX; last tie wins).

v2 layout: the whole kernel is a software pipeline around the DVE argmax
(the only engine that must stream every score).  Image b+1's prep
(downsample/unfold/squares/bias) is emitted interleaved with image b's
main loop so it runs on Pool/ACT/PE slack; tails are split in tile
ranges so the gather/L1 work overlaps the next image's main loop.  The
unfold shuffles write f32r staging tiles that DMA straight into rr
(no rh intermediate); prep DMAs are batched (2 const + ~10 per image).
"""

import sys

sys.path.insert(0, "/opt/trn_rl_repo")

import numpy as np

import concourse.bacc as bacc
import concourse.mybir as mybir
import concourse.tile as tile
from concourse.bass_utils import run_bass_kernel_spmd

# ---------------- problem constants (hardcoded) ----------------
B_FULL = 16
NCORES = 8
B_LOC = B_FULL // NCORES       # images per core
C, H, W = 3, 144, 144
G = 48                         # patch grid (144/3)
NI = G * G                     # 2304 query patches
D = 27                         # C*3*3
NQ = NI + (G // 2) ** 2 + (G // 4) ** 2  # 3024
KD = 33                        # contraction: 27 data + 5 zero + bias@32
KZ = 32                        # bias row partition (32-aligned)
JT = 504                       # j tile; 6 per row
NJT = NQ // JT                 # 6
IT = 128
NIT = NI // IT                 # 18
CUBIC_W = np.array([-0.09375, 0.59375, 0.59375, -0.09375], dtype=np.float32)

F32 = mybir.dt.float32
F32R = mybir.dt.float32r
I16 = mybir.dt.int16
ADD = mybir.AluOpType.add
SUB = mybir.AluOpType.subtract
MUL = mybir.AluOpType.mult
ABS = mybir.ActivationFunctionType.Abs
SQ = mybir.ActivationFunctionType.Square

# packA column layout
_CA_CD2, _CA_CD4, _CA_IDN, _CA_PMT, _CA_ONE = 0, 72, 108, 236, 1260
CA_COLS = 1261

# ---------------- custom DVE op: single-pass argmax -------------------
from concourse.dve_spec import Spec, Src0, Idx, MaxNeg, select, scan, AluOp, maxx, lower
from concourse.dve_uop import DveOpSpec
import concourse.dve_ops as dve_ops
from concourse.dve_ops import DveOp


def _argmax_ref(in0, in1, c0, c1, c2):
    run = np.maximum.accumulate(in0, axis=-1)
    out = np.where(in0 >= run, np.arange(in0.shape[-1], dtype=np.float32),
                   -np.finfo(np.float32).max)
    acc = out.reshape(out.shape[0], -1).max(axis=-1, keepdims=True)
    return out.astype(np.float32), acc.astype(np.float32)


def _register_argmax_op():
    name = "ANT_ARGMAX_LAST"
    if name in dve_ops._SUB_OPCODE_FOR_NAME:
        return next(op for op in dve_ops.OPS if op.name == name)
    body = select(Src0 >= scan(AluOp.MAX, Src0), Idx, MaxNeg)
    spec = Spec(body=body, accum=maxx, reference=_argmax_ref)
    opcode = dve_ops._CUSTOM_DVE_ROW_BASE + len(dve_ops.OPS)
    shas = {v: DveOpSpec(name=name, opcode=opcode, uops=lower(spec, ver=v),
                         rd1_en=False).sha(v) for v in ("v3", "v4")}
    op = DveOp(name, spec, subdim=False, uops_sha=shas)
    dve_ops.OPS.append(op)
    dve_ops._SUB_OPCODE_FOR_NAME[name] = opcode
    dve_ops.CUSTOM_DVE_SPECS[name] = spec
    return op


ARGMAX_OP = _register_argmax_op()

# ---------------- host-side constants ---------------------------------


def _down_matrix(n, f):
    """M[h, i]: out[i] = sum_h M[h, i] * in[h]  (torch bicubic, offset t=.5)."""
    out_n = n // f
    M = np.zeros((n, out_n), dtype=np.float32)
    for i in range(out_n):
        base = f * i + (f // 2 - 1)
        for a in range(4):
            h = min(max(base + a - 1, 0), n - 1)
            M[h, i] += CUBIC_W[a]
    return M


def _perm_matrices():
    """PMT[:, m*128 + r]: one-hot at row (m*16 + r%16) -> out_m = Pm @ v."""
    P = np.zeros((128, 8 * 128), dtype=np.float32)
    for m in range(8):
        for r in range(128):
            P[m * 16 + r % 16, m * 128 + r] = 1.0
    return P


def make_consts():
    cd2 = _down_matrix(H, 2)   # [144, 72]
    cd4 = _down_matrix(H, 4)   # [144, 36]
    packA = np.zeros((128, CA_COLS), dtype=np.float32)
    packA[:, _CA_CD2:_CA_CD2 + 72] = cd2[0:128]
    packA[:, _CA_CD4:_CA_CD4 + 36] = cd4[0:128]
    packA[:, _CA_IDN:_CA_IDN + 128] = np.eye(128, dtype=np.float32)
    packA[:, _CA_PMT:_CA_PMT + 1024] = _perm_matrices()
    packA[:, _CA_ONE] = 1.0
    packB = np.zeros((16, 108), dtype=np.float32)
    packB[:, 0:72] = cd2[128:144]
    packB[:, 72:108] = cd4[128:144]
    return {"packA": np.ascontiguousarray(packA),
            "packB": np.ascontiguousarray(packB)}


# ---------------- kernel construction ---------------------------------


def build_nc(debug=False):
    nc = bacc.Bacc("TRN2", target_bir_lowering=False)

    x_d = nc.dram_tensor("x", [B_LOC, C, H, W], F32, kind="ExternalInput")
    gt_d = nc.dram_tensor("gt", [B_LOC, C, H, W], F32, kind="ExternalInput")
    pa_d = nc.dram_tensor("packA", [128, CA_COLS], F32, kind="ExternalInput")
    pb_d = nc.dram_tensor("packB", [16, 108], F32, kind="ExternalInput")
    d2_d = nc.dram_tensor("scr_d2", [C, 72, 72], F32, kind="Internal")
    d4_d = nc.dram_tensor("scr_d4", [C, 36, 36], F32, kind="Internal")
    loss_d = nc.dram_tensor("loss", [1, 1], F32, kind="ExternalOutput")
    dbg = {}
    if debug:
        dbg["idxf"] = nc.dram_tensor("dbg_idxf", [B_LOC, 128, NIT], F32,
                                     kind="ExternalOutput")
        dbg["rr"] = nc.dram_tensor("dbg_rr", [B_LOC, KD, NQ], F32,
                                   kind="ExternalOutput")
        dbg["p1t"] = nc.dram_tensor("dbg_p1t", [B_LOC, D, NI], F32,
                                    kind="ExternalOutput")
        dbg["lr"] = nc.dram_tensor("dbg_lr", [B_LOC, KD, NI], F32,
                                   kind="ExternalOutput")
        dbg["part"] = nc.dram_tensor("dbg_part", [D, 2 * B_LOC], F32,
                                     kind="ExternalOutput")

    with tile.TileContext(nc) as tc:
        with (
            tc.tile_pool(name="consts", bufs=1) as cpool,
            tc.tile_pool(name="persist", bufs=1) as ppool,   # per-image slots
            tc.tile_pool(name="stage", bufs=1) as stpool,    # unfold staging
            tc.tile_pool(name="dsw", bufs=1) as dpool,       # downsample work
            tc.tile_pool(name="score", bufs=2) as scpool,
            tc.tile_pool(name="small", bufs=2) as smpool,
            tc.tile_pool(name="psmain", bufs=2, space="PSUM") as psm,
            tc.tile_pool(name="pssmall", bufs=2, space="PSUM") as pss,
        ):
            # ---- consts ----
            pa = cpool.tile([128, CA_COLS], F32, tag="pa")
            pb = cpool.tile([16, 108], F32, tag="pb")
            nc.sync.dma_start(pa[:], pa_d[:])
            nc.sync.dma_start(pb[:], pb_d[:])
            cd2a = pa[:, _CA_CD2:_CA_CD2 + 72]
            cd4a = pa[:, _CA_CD4:_CA_CD4 + 36]
            idn = pa[:, _CA_IDN:_CA_IDN + 128]
            pmt = pa[:, _CA_PMT:_CA_PMT + 1024]
            ones = pa[:, _CA_ONE:_CA_ONE + 1]
            cd2b = pb[:, 0:72]
            cd4b = pb[:, 72:108]

            ngo = cpool.tile([D, 1], F32R, tag="ngo")
            nc.gpsimd.memset(ngo[:].bitcast(F32), -1.0)

            junk = cpool.tile([128, NQ], F32, tag="junk")
            junkd = cpool.tile([D, IT * 9], F32, tag="junkd")
            part = cpool.tile([D, 5], F32, tag="part")

            # ---- per-image persistent slots ----
            rr = [ppool.tile([KD, NQ], F32R, tag=f"rr{b}", name=f"rr{b}")
                  for b in range(B_LOC)]
            lr = [ppool.tile([KD, NI], F32R, tag=f"lr{b}", name=f"lr{b}")
                  for b in range(B_LOC)]
            p1t = [ppool.tile([D, NI], F32R, tag=f"p1t{b}", name=f"p1t{b}")
                   for b in range(B_LOC)]
            qsq_t = ppool.tile([D, NQ], F32R, tag="qsq", name="qsq")
            qsq = [qsq_t, qsq_t]  # shared; image1 write WARs image0's reads
            idxf = [ppool.tile([128, NIT], F32, tag=f"idxf{b}", name=f"idxf{b}")
                    for b in range(B_LOC)]
            for b in range(B_LOC):
                # zero rows 0..32 once per slot (engine partition windows are
                # 32-aligned); unfold DMAs / TT adds overwrite rows 0..27
                # later, rows 27..32 stay zero, lr bias row 32 = +1.
                nc.gpsimd.memset(rr[b][0:KZ, :].bitcast(F32), 0.0)
                nc.gpsimd.memset(lr[b][0:KZ, :].bitcast(F32), 0.0)
                nc.gpsimd.memset(lr[b][KZ:KD, :].bitcast(F32), 1.0)

            # ---- unfold staging tiles ----
            at = [stpool.tile([9, 3456], F32, tag=f"at{h}", name=f"at{h}")
                  for h in range(2)]
            bt_t = stpool.tile([9, 3456], F32R, tag="bt", name="bt")
            bt = [bt_t, bt_t]
            dstg = stpool.tile([9, 1728], F32, tag="dstg")
            dbt = stpool.tile([9, 1728], F32R, tag="dbt")

            # ---- downsample work tiles ----
            ga = dpool.tile([128, C * W], F32, tag="ga")
            gb = dpool.tile([16, C * W], F32, tag="gb")
            gh = dpool.tile([72, C * W], F32, tag="gh")
            ghta = dpool.tile([128, C * 72], F32, tag="ghta")
            ghtb = dpool.tile([16, C * 72], F32, tag="ghtb")
            g2 = dpool.tile([72, C * 72], F32, tag="g2")     # d2 image
            g4 = dpool.tile([36, C * 36], F32, tag="g4")     # d4 image

            def downsample(f, dst, E):
                """ga/gb [128/16, C*W] SBUF --bicubic/f--> dst [n, C*n]."""
                n = H // f
                cda = cd2a if f == 2 else cd4a
                cdb = cd2b if f == 2 else cd4b
                ghp = pss.tile([128, 512], F32, tag="ps", name="ghp")
                nc.tensor.matmul(ghp[0:n, 0:C * W], cda[:, 0:n], ga[:],
                                 start=True, stop=False)
                nc.tensor.matmul(ghp[0:n, 0:C * W], cdb[:, 0:n], gb[:],
                                 start=False, stop=True)
                E.copy(gh[0:n, :], ghp[0:n, 0:C * W])
                gh3 = gh[:].rearrange("i (c w) -> i c w", c=C)
                tpa = pss.tile([128, 512], F32, tag="ps", name="tpa")
                tpb = pss.tile([128, 512], F32, tag="ps", name="tpb")
                for c in range(C):
                    nc.tensor.transpose(tpa[0:128, c * n:(c + 1) * n],
                                        gh3[0:n, c, 0:128], idn[0:n, 0:n])
                    nc.tensor.transpose(tpb[0:16, c * n:(c + 1) * n],
                                        gh3[0:n, c, 128:144], idn[0:n, 0:n])
                E.copy(ghta[:, 0:C * n], tpa[0:128, 0:C * n])
                E.copy(ghtb[:, 0:C * n], tpb[0:16, 0:C * n])
                ghta3 = ghta[:].rearrange("w (c i) -> w c i", c=C)
                ghtb3 = ghtb[:].rearrange("w (c i) -> w c i", c=C)
                op = pss.tile([128, 512], F32, tag="ps", name="op")
                for c in range(C):
                    nc.tensor.matmul(op[0:n, c * n:(c + 1) * n],
                                     ghta3[:, c, 0:n], cda[:, 0:n],
                                     start=True, stop=False)
                    nc.tensor.matmul(op[0:n, c * n:(c + 1) * n],
                                     ghtb3[:, c, 0:n], cdb[:, 0:n],
                                     start=False, stop=True)
                E.copy(dst[:, :], op[0:n, 0:C * n])

            def unfold_half(b, src_dram, hf, dst_ap, E):
                """Full-res unfold half hf: 24 of 48 gi rows -> dst [27, 1152]."""
                a = at[hf]
                src = src_dram.rearrange("c (gi r) w -> c r gi w", r=3)
                for c in range(C):
                    nc.sync.dma_start(
                        a[3 * c:3 * c + 3, :],
                        src[c, :, hf * 24:(hf + 1) * 24, :])
                o = bt[hf]
                av = a[:].rearrange("p (gi gj s) -> p s gi gj", gi=24, gj=48)
                ov = o[:].rearrange("p (s gi gj) -> p s gi gj", s=3, gi=24)
                E.tensor_copy(ov, av)
                nc.sync.dma_start(dst_ap, o[:])

            def unfold_small(b, img, n, dst_ap, E):
                """Unfold of img [n, C*n] (n=72 d2 / 36 d4) via a DRAM bounce
                (the (gi r)->(c r) partition regroup isn't one DMA)."""
                g = n // 3
                sz = g * n                       # per-channel elements
                scr = d2_d if n == 72 else d4_d
                nc.sync.dma_start(scr.rearrange("c i j -> i c j"), img[:])
                for c in range(C):
                    nc.sync.dma_start(
                        dstg[3 * c:3 * c + 3, 0:sz],
                        scr[c].rearrange("(gi r) j -> r gi j", r=3))
                av = dstg[:, 0:sz].rearrange("p (gi gj s) -> p s gi gj",
                                             gi=g, gj=g)
                ov = dbt[:, 0:sz].rearrange("p (s gi gj) -> p s gi gj",
                                            s=3, gi=g)
                E.tensor_copy(ov, av)
                nc.sync.dma_start(dst_ap, dbt[:, 0:sz])

            def squares(b, lo, hi, E):
                if E is nc.scalar:
                    nc.scalar.activation(qsq[b][:, lo:hi],
                                         rr[b][0:D, lo:hi].bitcast(F32), SQ)
                else:
                    E.tensor_tensor(qsq[b][:, lo:hi],
                                    rr[b][0:D, lo:hi].bitcast(F32),
                                    rr[b][0:D, lo:hi].bitcast(F32), op=MUL)

            def bias_chunk(b, jt):
                bnp = pss.tile([128, 512], F32, tag="ps", name="bnp")
                nc.tensor.matmul(bnp[0:1, 0:JT], ngo[:],
                                 qsq[b][:, jt * JT:(jt + 1) * JT])
                nc.scalar.copy(rr[b][KZ:KZ + 1, jt * JT:(jt + 1) * JT],
                               bnp[0:1, 0:JT])

            def load_ds_src(b):
                gsrc = gt_d[b].rearrange("c h w -> h c w")
                nc.sync.dma_start(ga[:].rearrange("h (c w) -> h c w", c=C),
                                  gsrc[0:128])
                nc.sync.dma_start(gb[:].rearrange("h (c w) -> h c w", c=C),
                                  gsrc[128:144])

            def lr_fill(b, E):
                E.tensor_tensor(lr[b][0:D, :], p1t[b][:, :].bitcast(F32),
                                rr[b][0:D, 0:NI].bitcast(F32), op=ADD)

            def main_tile(b, t):
                sc = scpool.tile([128, NQ], F32, tag="sc")
                for third in range(2):
                    ps = psm.tile([128, 3, 512], F32, tag="psmain")
                    for k in range(3):
                        jt = third * 3 + k
                        nc.tensor.matmul(
                            ps[:, k, 0:JT],
                            lr[b][:, t * IT:(t + 1) * IT],
                            rr[b][:, jt * JT:(jt + 1) * JT],
                        )
                    nc.scalar.copy(
                        sc[:, third * 3 * JT:(third + 1) * 3 * JT],
                        ps[:, :, 0:JT],
                    )
                nc.vector._custom_dve(
                    ARGMAX_OP, out=junk[:], in0=sc[:],
                    accum_out=idxf[b][:, t:t + 1],
                )

            def tail_range(b, tlo, thi, col, df_eng):
                """Gather + L1 for i-tiles [tlo, thi) -> part[:, col]."""
                nt = thi - tlo
                wp = pss.tile([128, 512], F32, tag="ps", name="wp")
                wp3 = wp[0:128, 0:8 * nt].rearrange("p (m t) -> p m t", m=8)
                for m in range(8):
                    nc.tensor.matmul(wp3[:, m, :],
                                     pmt[:, m * 128:(m + 1) * 128],
                                     idxf[b][:, tlo:thi])
                widx = smpool.tile([128, 8 * 9], I16, tag="widx")
                w3 = widx[:, 0:8 * nt].rearrange("p (t m) -> p t m", t=nt)
                nc.vector.tensor_copy(w3[:, :, :],
                                      wp3[:, :, :].rearrange("p m t -> p t m"))
                sel = smpool.tile([32, IT * 9], F32, tag="sel", bufs=1)
                ni = IT * nt
                nc.gpsimd.ap_gather(
                    sel[:, 0:ni].rearrange("p (n d) -> p n d", d=1),
                    rr[b][0:32, :].bitcast(F32).rearrange(
                        "p (n d) -> p n d", d=1),
                    widx[0:32, 0:8 * nt],
                    channels=32, num_elems=NQ, d=1, num_idxs=ni,
                )
                df = smpool.tile([D, IT * 9], F32, tag="df", bufs=1)
                df_eng.tensor_tensor(df[:, 0:ni], sel[0:D, 0:ni],
                                     p1t[b][:, tlo * IT:thi * IT].bitcast(F32),
                                     op=SUB)
                nc.scalar.activation(junkd[:, 0:ni], df[:, 0:ni], ABS,
                                     accum_out=part[0:D, col:col + 1])

            # ================= schedule =================
            # ---- prep image 0 (exposed; spread across engines) ----
            load_ds_src(0)
            downsample(2, g2, nc.scalar)
            unfold_half(0, gt_d[0], 0, rr[0][0:D, 0:1152], nc.vector)
            unfold_half(0, gt_d[0], 1, rr[0][0:D, 1152:NI], nc.vector)
            unfold_half(0, x_d[0], 0, p1t[0][:, 0:1152], nc.vector)
            unfold_half(0, x_d[0], 1, p1t[0][:, 1152:NI], nc.vector)
            downsample(4, g4, nc.scalar)
            squares(0, 0, NI, nc.scalar)
            for jt in range(4):
                bias_chunk(0, jt)
            unfold_small(0, g2, 72, rr[0][0:D, NI:NI + 576], nc.gpsimd)
            unfold_small(0, g4, 36, rr[0][0:D, NI + 576:NQ], nc.gpsimd)
            lr_fill(0, nc.vector)
            squares(0, NI, NQ, nc.scalar)
            for jt in range(4, NJT):
                bias_chunk(0, jt)

            # ---- main(0) with prep(1) interleaved ----
            for t in range(NIT):
                main_tile(0, t)
                if t == 0:
                    load_ds_src(1)
                elif t == 1:
                    downsample(2, g2, nc.scalar)
                elif t == 2:
                    downsample(4, g4, nc.scalar)
                elif t == 3:
                    unfold_half(1, gt_d[1], 0, rr[1][0:D, 0:1152], nc.gpsimd)
                elif t == 4:
                    unfold_half(1, gt_d[1], 1, rr[1][0:D, 1152:NI], nc.gpsimd)
                elif t == 5:
                    unfold_small(1, g2, 72, rr[1][0:D, NI:NI + 576], nc.gpsimd)
                    unfold_small(1, g4, 36, rr[1][0:D, NI + 576:NQ], nc.gpsimd)
                elif t == 6:
                    unfold_half(1, x_d[1], 0, p1t[1][:, 0:1152], nc.gpsimd)
                elif t == 7:
                    unfold_half(1, x_d[1], 1, p1t[1][:, 1152:NI], nc.gpsimd)
                elif t == 8:
                    squares(1, 0, NI, nc.gpsimd)
                elif t == 9:
                    squares(1, NI, NQ, nc.gpsimd)
                    lr_fill(1, nc.gpsimd)
                elif t == 10:
                    for jt in range(NJT):
                        bias_chunk(1, jt)
                elif t == 11:
                    tail_range(0, 0, 9, 0, nc.gpsimd)

            # ---- main(1); image-0 tail B then image-1 tails ----
            for t in range(NIT):
                main_tile(1, t)
                if t == 0:
                    tail_range(0, 9, NIT, 1, nc.gpsimd)
                elif t == 11:
                    tail_range(1, 0, 9, 2, nc.gpsimd)
                elif t == 15:
                    tail_range(1, 9, 15, 3, nc.gpsimd)
            tail_range(1, 15, NIT, 4, nc.vector)

            # ---- final loss ----
            fin = pss.tile([128, 512], F32, tag="ps", name="fin")
            nc.tensor.matmul(fin[0:1, 0:5], ones[0:D, :], part[0:D, 0:5])
            lt = smpool.tile([1, 1], F32, tag="loss")
            ltj = smpool.tile([1, 5], F32, tag="ltj")
            nc.scalar.activation(ltj[:], fin[0:1, 0:5], ABS,
                                 accum_out=lt[:])
            nc.sync.dma_start(loss_d[:], lt[:])

            if debug:
                for b in range(B_LOC):
                    nc.sync.dma_start(dbg["idxf"][b], idxf[b][:, :])
                    nc.sync.dma_start(dbg["rr"][b], rr[b][:, :].bitcast(F32))
                    nc.sync.dma_start(dbg["p1t"][b], p1t[b][:, :].bitcast(F32))
                    nc.sync.dma_start(dbg["lr"][b], lr[b][:, :].bitcast(F32))
                nc.sync.dma_start(dbg["part"][:, :], part[:, :])

    nc.compile()
    return nc


_NC_CACHE = None


def _get_nc():
    global _NC_CACHE
    if _NC_CACHE is None:
        _NC_CACHE = build_nc()
    return _NC_CACHE


# part column layout: (b, range) -> col
PART_COLS = 5  # [b0 t0:9, b0 t9:18, b1 t0:9, b1 t9:15, b1 t15:18]


def kernel(x: np.ndarray, gt: np.ndarray, _trace=False, _debug=False):
    x = np.ascontiguousarray(np.asarray(x, dtype=np.float32))
    gt = np.ascontiguousarray(np.asarray(gt, dtype=np.float32))
    consts = make_consts()
    nc = build_nc(debug=True) if _debug else _get_nc()
    in_maps = []
    for c in range(NCORES):
        m = {"x": x[c * B_LOC:(c + 1) * B_LOC],
             "gt": gt[c * B_LOC:(c + 1) * B_LOC]}
        m.update(consts)
        in_maps.append(m)
    res = run_bass_kernel_spmd(
        nc, in_maps, core_ids=list(range(NCORES)), trace=_trace,
        trace_cores=[0] if _trace else None,
    )
    total = sum(float(r["loss"][0, 0]) for r in res.results)
    out = np.asarray(np.float32(total / (B_FULL * NI * D)))
    if _debug or _trace:
        return out, res
    return out


if __name__ == "__main__":
    xs = np.load("/root/problem/work/x.npy")
    gts = np.load("/root/problem/work/gt.npy")
    expected = float(np.load("/root/problem/work/expected.npy"))
    got = float(kernel(xs, gts))
    rel = abs(got - expected) / abs(expected)
    print(f"expected {expected:.8f}  got {got:.8f}  relerr {rel:.3e}")


# revision 54
# speedup vs baseline: 1.2577x; 1.1188x over previous
"""BestBuddyLoss Trainium2 kernel (8-core data parallel).

Per image: p1 = unfold(x), p2 = unfold(gt),
q = concat(p2, unfold(down2(gt)), unfold(down4(gt)))  -> [3024, 27].
score(i,j) = d(p1_i,q_j) + d(p2_i,q_j) (squared-L2/d, clamped at 0).
With alpha=beta=1 the clamp/scale don't change the argmin:
  argmin_j score == argmax_j ( <p1_i + p2_i, q_j> - |q_j|^2 )
so each (i,j) tile is one K=33 f32r matmul with lhsT=[(p1+p2)^T; ones],
rhs=[q^T; -|q|^2].  The argmax over j is a single-pass fused custom DVE
op (running-max scan + select(Idx) + accum MAX; last tie wins).

v2 layout: the whole kernel is a software pipeline around the DVE argmax
(the only engine that must stream every score).  Image b+1's prep
(downsample/unfold/squares/bias) is emitted interleaved with image b's
main loop so it runs on Pool/ACT/PE slack; tails are split in tile
ranges so the gather/L1 work overlaps the next image's main loop.  The
unfold shuffles write f32r staging tiles that DMA straight into rr
(no rh intermediate); prep DMAs are batched (2 const + ~10 per image).
"""

import sys

sys.path.insert(0, "/opt/trn_rl_repo")

import numpy as np

import concourse.bacc as bacc
import concourse.mybir as mybir
import concourse.tile as tile
from concourse.bass_utils import run_bass_kernel_spmd

# ---------------- problem constants (hardcoded) ----------------
B_FULL = 16
NCORES = 8
B_LOC = B_FULL // NCORES       # images per core
C, H, W = 3, 144, 144
G = 48                         # patch grid (144/3)
NI = G * G                     # 2304 query patches
D = 27                         # C*3*3
NQ = NI + (G // 2) ** 2 + (G // 4) ** 2  # 3024
KD = 33                        # contraction: 27 data + 5 zero + bias@32
KZ = 32                        # bias row partition (32-aligned)
JT = 504                       # j tile; 6 per row
NJT = NQ // JT                 # 6
IT = 128
NIT = NI // IT                 # 18
CUBIC_W = np.array([-0.09375, 0.59375, 0.59375, -0.09375], dtype=np.float32)

F32 = mybir.dt.float32
F32R = mybir.dt.float32r
I16 = mybir.dt.int16
ADD = mybir.AluOpType.add
SUB = mybir.AluOpType.subtract
MUL = mybir.AluOpType.mult
ABS = mybir.ActivationFunctionType.Abs
SQ = mybir.ActivationFunctionType.Square

# packA column layout
_CA_CD2, _CA_CD4, _CA_IDN, _CA_PMT, _CA_ONE = 0, 72, 108, 236, 1260
CA_COLS = 1261

# ---------------- custom DVE op: single-pass argmax -------------------
from concourse.dve_spec import Spec, Src0, Idx, MaxNeg, select, scan, AluOp, maxx, lower
from concourse.dve_uop import DveOpSpec
import concourse.dve_ops as dve_ops
from concourse.dve_ops import DveOp


def _argmax_ref(in0, in1, c0, c1, c2):
    run = np.maximum.accumulate(in0, axis=-1)
    out = np.where(in0 >= run, np.arange(in0.shape[-1], dtype=np.float32),
                   -np.finfo(np.float32).max)
    acc = out.reshape(out.shape[0], -1).max(axis=-1, keepdims=True)
    return out.astype(np.float32), acc.astype(np.float32)


def _register_argmax_op():
    name = "ANT_ARGMAX_LAST"
    if name in dve_ops._SUB_OPCODE_FOR_NAME:
        return next(op for op in dve_ops.OPS if op.name == name)
    body = select(Src0 >= scan(AluOp.MAX, Src0), Idx, MaxNeg)
    spec = Spec(body=body, accum=maxx, reference=_argmax_ref)
    opcode = dve_ops._CUSTOM_DVE_ROW_BASE + len(dve_ops.OPS)
    shas = {v: DveOpSpec(name=name, opcode=opcode, uops=lower(spec, ver=v),
                         rd1_en=False).sha(v) for v in ("v3", "v4")}
    op = DveOp(name, spec, subdim=False, uops_sha=shas)
    dve_ops.OPS.append(op)
    dve_ops._SUB_OPCODE_FOR_NAME[name] = opcode
    dve_ops.CUSTOM_DVE_SPECS[name] = spec
    return op


ARGMAX_OP = _register_argmax_op()

# ---------------- host-side constants ---------------------------------


def _down_matrix(n, f):
    """M[h, i]: out[i] = sum_h M[h, i] * in[h]  (torch bicubic, offset t=.5)."""
    out_n = n // f
    M = np.zeros((n, out_n), dtype=np.float32)
    for i in range(out_n):
        base = f * i + (f // 2 - 1)
        for a in range(4):
            h = min(max(base + a - 1, 0), n - 1)
            M[h, i] += CUBIC_W[a]
    return M


def _perm_matrices():
    """PMT[:, m*128 + r]: one-hot at row (m*16 + r%16) -> out_m = Pm @ v."""
    P = np.zeros((128, 8 * 128), dtype=np.float32)
    for m in range(8):
        for r in range(128):
            P[m * 16 + r % 16, m * 128 + r] = 1.0
    return P


def make_consts():
    cd2 = _down_matrix(H, 2)   # [144, 72]
    cd4 = _down_matrix(H, 4)   # [144, 36]
    packA = np.zeros((128, CA_COLS), dtype=np.float32)
    packA[:, _CA_CD2:_CA_CD2 + 72] = cd2[0:128]
    packA[:, _CA_CD4:_CA_CD4 + 36] = cd4[0:128]
    packA[:, _CA_IDN:_CA_IDN + 128] = np.eye(128, dtype=np.float32)
    packA[:, _CA_PMT:_CA_PMT + 1024] = _perm_matrices()
    packA[:, _CA_ONE] = 1.0
    packB = np.zeros((16, 108), dtype=np.float32)
    packB[:, 0:72] = cd2[128:144]
    packB[:, 72:108] = cd4[128:144]
    padrows = np.zeros((6, NQ), dtype=np.float32)
    padrows[5, :] = 1.0
    return {"packA": np.ascontiguousarray(packA),
            "packB": np.ascontiguousarray(packB),
            "padrows": padrows}


# ---------------- kernel construction ---------------------------------


def build_nc(debug=False):
    nc = bacc.Bacc("TRN2", target_bir_lowering=False)

    x_d = nc.dram_tensor("x", [B_LOC, C, H, W], F32, kind="ExternalInput")
    gt_d = nc.dram_tensor("gt", [B_LOC, C, H, W], F32, kind="ExternalInput")
    pa_d = nc.dram_tensor("packA", [128, CA_COLS], F32, kind="ExternalInput")
    pb_d = nc.dram_tensor("packB", [16, 108], F32, kind="ExternalInput")
    pr_d = nc.dram_tensor("padrows", [6, NQ], F32, kind="ExternalInput")
    d2_d = nc.dram_tensor("scr_d2", [C, 72, 72], F32, kind="Internal")
    d4_d = nc.dram_tensor("scr_d4", [C, 36, 36], F32, kind="Internal")
    loss_d = nc.dram_tensor("loss", [1, 1], F32, kind="ExternalOutput")
    dbg = {}
    if debug:
        dbg["idxf"] = nc.dram_tensor("dbg_idxf", [B_LOC, 128, NIT], F32,
                                     kind="ExternalOutput")
        dbg["rr"] = nc.dram_tensor("dbg_rr", [B_LOC, KD, NQ], F32,
                                   kind="ExternalOutput")
        dbg["p1t"] = nc.dram_tensor("dbg_p1t", [B_LOC, D, NI], F32,
                                    kind="ExternalOutput")
        dbg["lr"] = nc.dram_tensor("dbg_lr", [B_LOC, KD, NI], F32,
                                   kind="ExternalOutput")
        dbg["part"] = nc.dram_tensor("dbg_part", [D, 2 * B_LOC], F32,
                                     kind="ExternalOutput")

    with tile.TileContext(nc) as tc:
        with (
            tc.tile_pool(name="consts", bufs=1) as cpool,
            tc.tile_pool(name="persist", bufs=1) as ppool,   # per-image slots
            tc.tile_pool(name="stage", bufs=1) as stpool,    # unfold staging
            tc.tile_pool(name="dsw", bufs=1) as dpool,       # downsample work
            tc.tile_pool(name="score", bufs=2) as scpool,
            tc.tile_pool(name="small", bufs=2) as smpool,
            tc.tile_pool(name="psmain", bufs=2, space="PSUM") as psm,
            tc.tile_pool(name="pssmall", bufs=2, space="PSUM") as pss,
        ):
            # ---- consts ----
            pa = cpool.tile([128, CA_COLS], F32, tag="pa")
            pb = cpool.tile([16, 108], F32, tag="pb")
            nc.scalar.dma_start(pa[:], pa_d[:])
            nc.scalar.dma_start(pb[:], pb_d[:])
            cd2a = pa[:, _CA_CD2:_CA_CD2 + 72]
            cd4a = pa[:, _CA_CD4:_CA_CD4 + 36]
            idn = pa[:, _CA_IDN:_CA_IDN + 128]
            pmt = pa[:, _CA_PMT:_CA_PMT + 1024]
            ones = pa[:, _CA_ONE:_CA_ONE + 1]
            cd2b = pb[:, 0:72]
            cd4b = pb[:, 72:108]

            ngo = cpool.tile([D, 1], F32R, tag="ngo")
            nc.gpsimd.memset(ngo[:].bitcast(F32), -1.0)

            junk = cpool.tile([128, NQ], F32, tag="junk")
            junkd = cpool.tile([D, IT * 9], F32, tag="junkd")
            part = cpool.tile([D, 5], F32, tag="part")

            # ---- per-image persistent slots ----
            rr = [ppool.tile([KD, NQ], F32R, tag=f"rr{b}", name=f"rr{b}")
                  for b in range(B_LOC)]
            lr = [ppool.tile([KD, NI], F32R, tag=f"lr{b}", name=f"lr{b}")
                  for b in range(B_LOC)]
            qsq_t = ppool.tile([D, NQ], F32R, tag="qsq", name="qsq")
            qsq = [qsq_t, qsq_t]  # shared; image1 write WARs image0's reads
            idxf = [ppool.tile([128, NIT], F32, tag=f"idxf{b}", name=f"idxf{b}")
                    for b in range(B_LOC)]
            # pad rows (rr 27..32 zero; lr 27..32 zero + bias row 32 = +1)
            # filled once per slot via SWDGE casting DMAs from a DRAM const:
            # zero engine time, and no WAW against the row 0..27 writers.
            for b in range(B_LOC):
                nc.gpsimd.dma_start(rr[b][D:KZ, :], pr_d[0:5, :])
                nc.gpsimd.dma_start(lr[b][D:KD, :], pr_d[0:6, 0:NI])

            # ---- unfold staging tiles: two independent pairs (gt / x
            # paths); d2/d4 unfolds reuse slices of them ----
            at = [stpool.tile([9, 3456], F32, tag=f"at{h}", name=f"at{h}")
                  for h in range(2)]
            bt = [stpool.tile([9, 3456], F32R, tag=f"bt{h}", name=f"bt{h}")
                  for h in range(2)]
            dstg = stpool.tile([9, 1728], F32, tag="dstg")
            dbt = stpool.tile([9, 1728], F32R, tag="dbt")

            # ---- downsample work tiles ----
            ga = dpool.tile([128, C * W], F32, tag="ga")
            gb = dpool.tile([16, C * W], F32, tag="gb")
            gh = dpool.tile([72, C * W], F32, tag="gh")
            ghta = dpool.tile([128, C * 72], F32, tag="ghta")
            ghtb = dpool.tile([16, C * 72], F32, tag="ghtb")
            g2 = dpool.tile([72, C * 72], F32, tag="g2")     # d2 image
            g4 = dpool.tile([36, C * 36], F32, tag="g4")     # d4 image

            def downsample(f, dst, E):
                """ga/gb [128/16, C*W] SBUF --bicubic/f--> dst [n, C*n]."""
                n = H // f
                cda = cd2a if f == 2 else cd4a
                cdb = cd2b if f == 2 else cd4b
                ghp = pss.tile([128, 512], F32, tag="ps", name="ghp")
                nc.tensor.matmul(ghp[0:n, 0:C * W], cda[:, 0:n], ga[:],
                                 start=True, stop=False)
                nc.tensor.matmul(ghp[0:n, 0:C * W], cdb[:, 0:n], gb[:],
                                 start=False, stop=True)
                E.copy(gh[0:n, :], ghp[0:n, 0:C * W])
                gh3 = gh[:].rearrange("i (c w) -> i c w", c=C)
                tpa = pss.tile([128, 512], F32, tag="ps", name="tpa")
                tpb = pss.tile([128, 512], F32, tag="ps", name="tpb")
                for c in range(C):
                    nc.tensor.transpose(tpa[0:128, c * n:(c + 1) * n],
                                        gh3[0:n, c, 0:128], idn[0:n, 0:n])
                    nc.tensor.transpose(tpb[0:16, c * n:(c + 1) * n],
                                        gh3[0:n, c, 128:144], idn[0:n, 0:n])
                E.copy(ghta[:, 0:C * n], tpa[0:128, 0:C * n])
                E.copy(ghtb[:, 0:C * n], tpb[0:16, 0:C * n])
                ghta3 = ghta[:].rearrange("w (c i) -> w c i", c=C)
                ghtb3 = ghtb[:].rearrange("w (c i) -> w c i", c=C)
                op = pss.tile([128, 512], F32, tag="ps", name="op")
                for c in range(C):
                    nc.tensor.matmul(op[0:n, c * n:(c + 1) * n],
                                     ghta3[:, c, 0:n], cda[:, 0:n],
                                     start=True, stop=False)
                    nc.tensor.matmul(op[0:n, c * n:(c + 1) * n],
                                     ghtb3[:, c, 0:n], cdb[:, 0:n],
                                     start=False, stop=True)
                E.copy(dst[:, :], op[0:n, 0:C * n])

            def unfold_in(src_dram, hf, pair):
                """Queue the 3 stage-in DMAs for one unfold half."""
                a = at[pair]
                src = src_dram.rearrange("c (gi r) w -> c r gi w", r=3)
                for c in range(C):
                    nc.sync.dma_start(
                        a[3 * c:3 * c + 3, :],
                        src[c, :, hf * 24:(hf + 1) * 24, :])

            def unfold_out(dst_ap, E, pair):
                """Shuffle the staged half and DMA it out to dst [27, 1152]."""
                a, o = at[pair], bt[pair]
                av = a[:].rearrange("p (gi gj s) -> p s gi gj", gi=24, gj=48)
                ov = o[:].rearrange("p (s gi gj) -> p s gi gj", s=3, gi=24)
                if E is nc.scalar:
                    E.copy(ov, av)
                else:
                    E.tensor_copy(ov, av)
                nc.sync.dma_start(dst_ap, o[:])

            def unfold_small(img, n, dst_ap, E):
                """Unfold of img [n, C*n] (n=72 d2 / 36 d4) via a DRAM bounce
                (the (gi r)->(c r) partition regroup isn't one DMA)."""
                g = n // 3
                sz = g * n                       # per-channel elements
                scr = d2_d if n == 72 else d4_d
                nc.sync.dma_start(scr.rearrange("c i j -> i c j"), img[:])
                for c in range(C):
                    nc.sync.dma_start(
                        dstg[3 * c:3 * c + 3, 0:sz],
                        scr[c].rearrange("(gi r) j -> r gi j", r=3))
                av = dstg[:, 0:sz].rearrange("p (gi gj s) -> p s gi gj",
                                             gi=g, gj=g)
                ov = dbt[:, 0:sz].rearrange("p (s gi gj) -> p s gi gj",
                                            s=3, gi=g)
                E.tensor_copy(ov, av)
                nc.sync.dma_start(dst_ap, dbt[:, 0:sz])

            def squares(b, lo, hi, E):
                if E is nc.scalar:
                    nc.scalar.activation(qsq[b][:, lo:hi],
                                         rr[b][0:D, lo:hi].bitcast(F32), SQ)
                else:
                    E.tensor_tensor(qsq[b][:, lo:hi],
                                    rr[b][0:D, lo:hi].bitcast(F32),
                                    rr[b][0:D, lo:hi].bitcast(F32), op=MUL)

            def bias_chunk(b, jt, E=None):
                bnp = pss.tile([128, 512], F32, tag="ps", name="bnp")
                nc.tensor.matmul(bnp[0:1, 0:JT], ngo[:],
                                 qsq[b][:, jt * JT:(jt + 1) * JT])
                dst = rr[b][KZ:KZ + 1, jt * JT:(jt + 1) * JT]
                if E is nc.vector:
                    E.tensor_copy(dst, bnp[0:1, 0:JT])
                else:
                    nc.scalar.copy(dst, bnp[0:1, 0:JT])

            def load_ds_src(b, issuer=None):
                issuer = issuer or nc.sync
                gsrc = gt_d[b].rearrange("c h w -> h c w")
                issuer.dma_start(ga[:].rearrange("h (c w) -> h c w", c=C),
                                 gsrc[0:128])
                issuer.dma_start(gb[:].rearrange("h (c w) -> h c w", c=C),
                                 gsrc[128:144])

            def lr_fill(b, E):
                # lr rows 0:27 hold p1 (from the x unfold); add p2 in place.
                E.tensor_tensor(lr[b][0:D, :], lr[b][0:D, :].bitcast(F32),
                                rr[b][0:D, 0:NI].bitcast(F32), op=ADD)

            def main_tile(b, t):
                sc = scpool.tile([128, NQ], F32, tag="sc")
                for third in range(2):
                    ps = psm.tile([128, 3, 512], F32, tag="psmain")
                    for k in range(3):
                        jt = third * 3 + k
                        nc.tensor.matmul(
                            ps[:, k, 0:JT],
                            lr[b][:, t * IT:(t + 1) * IT],
                            rr[b][:, jt * JT:(jt + 1) * JT],
                        )
                    nc.scalar.copy(
                        sc[:, third * 3 * JT:(third + 1) * 3 * JT],
                        ps[:, :, 0:JT],
                    )
                nc.vector._custom_dve(
                    ARGMAX_OP, out=junk[:], in0=sc[:],
                    accum_out=idxf[b][:, t:t + 1],
                )

            def tail_range(b, tlo, thi, col, df_eng):
                """Gather + L1 for i-tiles [tlo, thi) -> part[:, col]."""
                nt = thi - tlo
                wp = pss.tile([128, 512], F32, tag="ps", name="wp")
                wp3 = wp[0:128, 0:8 * nt].rearrange("p (m t) -> p m t", m=8)
                for m in range(8):
                    nc.tensor.matmul(wp3[:, m, :],
                                     pmt[:, m * 128:(m + 1) * 128],
                                     idxf[b][:, tlo:thi])
                widx = smpool.tile([128, 8 * 9], I16, tag="widx")
                w3 = widx[:, 0:8 * nt].rearrange("p (t m) -> p t m", t=nt)
                nc.vector.tensor_copy(w3[:, :, :],
                                      wp3[:, :, :].rearrange("p m t -> p t m"))
                sel = smpool.tile([32, IT * 9], F32, tag="sel", bufs=1)
                ni = IT * nt
                nc.gpsimd.ap_gather(
                    sel[:, 0:ni].rearrange("p (n d) -> p n d", d=1),
                    rr[b][0:32, :].bitcast(F32).rearrange(
                        "p (n d) -> p n d", d=1),
                    widx[0:32, 0:8 * nt],
                    channels=32, num_elems=NQ, d=1, num_idxs=ni,
                )
                # p1 = lr - p2, so |p1 - sel| = |lr - (p2 + sel)|
                df = smpool.tile([D, IT * 9], F32, tag="df", bufs=1)
                cols = slice(tlo * IT, thi * IT)
                df_eng.tensor_tensor(df[:, 0:ni], sel[0:D, 0:ni],
                                     rr[b][0:D, cols].bitcast(F32), op=ADD)
                df_eng.tensor_tensor(df[:, 0:ni], lr[b][0:D, cols].bitcast(F32),
                                     df[:, 0:ni], op=SUB)
                nc.scalar.activation(junkd[:, 0:ni], df[:, 0:ni], ABS,
                                     accum_out=part[0:D, col:col + 1])

            # ================= schedule =================
            # ---- prep image 0 (exposed; spread across engines).  All six
            # stage-in DMA groups are queued before any dependent out-DMA so
            # the in-order SP queue never head-blocks a load behind a
            # compute-dependent store. ----
            load_ds_src(0, nc.scalar)   # ACT queue: dep-free early loads
            downsample(4, g4, nc.scalar)
            # gt halves stage through BOTH pairs so their loads don't
            # serialize behind each other's shuffles; x reuses them after.
            unfold_in(gt_d[0], 0, 0)
            unfold_in(gt_d[0], 1, 1)
            unfold_out(rr[0][0:D, 0:1152], nc.vector, 0)
            unfold_in(x_d[0], 0, 0)
            unfold_out(rr[0][0:D, 1152:NI], nc.vector, 1)
            unfold_in(x_d[0], 1, 1)
            unfold_out(lr[0][0:D, 0:1152], nc.vector, 0)
            unfold_out(lr[0][0:D, 1152:NI], nc.vector, 1)
            downsample(2, g2, nc.scalar)
            squares(0, 0, NI, nc.scalar)
            for jt in range(4):
                bias_chunk(0, jt)
            unfold_small(g2, 72, rr[0][0:D, NI:NI + 576], nc.gpsimd)
            unfold_small(g4, 36, rr[0][0:D, NI + 576:NQ], nc.gpsimd)
            lr_fill(0, nc.vector)
            squares(0, NI, NI + 576, nc.scalar)   # d2 part
            squares(0, NI + 576, NQ, nc.scalar)   # d4 part
            for jt in range(4, NJT):
                bias_chunk(0, jt)

            from contextlib import contextmanager

            @contextmanager
            def low_prio(off=1 << 20):
                # deprioritized vs the main mm->copy->argmax chain: runs only
                # in engine slack, never wins a tie against main-loop work.
                old = tc.cur_priority
                tc.cur_priority = old + off
                try:
                    yield
                finally:
                    tc.cur_priority = old

            # ---- main(0) with prep(1) interleaved (image-1 prep runs on
            # Pool shuffles + ACT copy slack + PE slack) ----
            for t in range(NIT):
                main_tile(0, t)
                if t == 1:
                    with low_prio():
                        unfold_in(gt_d[1], 0, 0)
                        unfold_in(x_d[1], 0, 1)
                elif t == 2:
                    with low_prio():
                        load_ds_src(1)
                        unfold_out(rr[1][0:D, 0:1152], nc.gpsimd, 0)
                        unfold_in(gt_d[1], 1, 0)
                        unfold_out(lr[1][0:D, 0:1152], nc.gpsimd, 1)
                        unfold_in(x_d[1], 1, 1)
                elif t == 3:
                    with low_prio():
                        downsample(2, g2, nc.scalar)
                        unfold_out(rr[1][0:D, 1152:NI], nc.gpsimd, 0)
                        unfold_out(lr[1][0:D, 1152:NI], nc.gpsimd, 1)
                elif t == 4:
                    with low_prio():
                        downsample(4, g4, nc.scalar)
                        squares(1, 0, NI, nc.gpsimd)
                elif t == 5:
                    with low_prio():
                        unfold_small(g2, 72, rr[1][0:D, NI:NI + 576],
                                     nc.gpsimd)
                        unfold_small(g4, 36, rr[1][0:D, NI + 576:NQ],
                                     nc.gpsimd)
                elif t == 6:
                    with low_prio():
                        lr_fill(1, nc.gpsimd)
                        squares(1, NI, NQ, nc.gpsimd)
                elif t == 7:
                    with low_prio():
                        for jt in range(NJT):
                            bias_chunk(1, jt)
                elif t == 11:
                    with low_prio():
                        tail_range(0, 0, 9, 0, nc.gpsimd)

            # ---- main(1); image-0 tail B then image-1 tails ----
            for t in range(NIT):
                main_tile(1, t)
                if t == 0:
                    with low_prio():
                        tail_range(0, 9, NIT, 1, nc.gpsimd)
                elif t == 11:
                    with low_prio():
                        tail_range(1, 0, 9, 2, nc.gpsimd)
                elif t == 15:
                    with low_prio():
                        tail_range(1, 9, 15, 3, nc.gpsimd)
            tail_range(1, 15, NIT, 4, nc.vector)

            # ---- final loss ----
            fin = pss.tile([128, 512], F32, tag="ps", name="fin")
            nc.tensor.matmul(fin[0:1, 0:5], ones[0:D, :], part[0:D, 0:5])
            lt = smpool.tile([1, 1], F32, tag="loss")
            ltj = smpool.tile([1, 5], F32, tag="ltj")
            nc.scalar.activation(ltj[:], fin[0:1, 0:5], ABS,
                                 accum_out=lt[:])
            nc.sync.dma_start(loss_d[:], lt[:])

            if debug:
                for b in range(B_LOC):
                    nc.sync.dma_start(dbg["idxf"][b], idxf[b][:, :])
                    nc.sync.dma_start(dbg["rr"][b], rr[b][:, :].bitcast(F32))
                    nc.sync.dma_start(dbg["lr"][b], lr[b][:, :].bitcast(F32))
                nc.sync.dma_start(dbg["part"][:, :], part[:, :])

    nc.compile()
    return nc


_NC_CACHE = None


def _get_nc():
    global _NC_CACHE
    if _NC_CACHE is None:
        _NC_CACHE = build_nc()
    return _NC_CACHE


# part column layout: (b, range) -> col
PART_COLS = 5  # [b0 t0:9, b0 t9:18, b1 t0:9, b1 t9:15, b1 t15:18]


def kernel(x: np.ndarray, gt: np.ndarray, _trace=False, _debug=False):
    x = np.ascontiguousarray(np.asarray(x, dtype=np.float32))
    gt = np.ascontiguousarray(np.asarray(gt, dtype=np.float32))
    consts = make_consts()
    nc = build_nc(debug=True) if _debug else _get_nc()
    in_maps = []
    for c in range(NCORES):
        m = {"x": x[c * B_LOC:(c + 1) * B_LOC],
             "gt": gt[c * B_LOC:(c + 1) * B_LOC]}
        m.update(consts)
        in_maps.append(m)
    res = run_bass_kernel_spmd(
        nc, in_maps, core_ids=list(range(NCORES)), trace=_trace,
        trace_cores=[0] if _trace else None,
    )
    total = sum(float(r["loss"][0, 0]) for r in res.results)
    out = np.asarray(np.float32(total / (B_FULL * NI * D)))
    if _debug or _trace:
        return out, res
    return out


if __name__ == "__main__":
    xs = np.load("/root/problem/work/x.npy")
    gts = np.load("/root/problem/work/gt.npy")
    expected = float(np.load("/root/problem/work/expected.npy"))
    got = float(kernel(xs, gts))
    rel = abs(got - expected) / abs(expected)
    print(f"expected {expected:.8f}  got {got:.8f}  relerr {rel:.3e}")


# revision 58
# speedup vs baseline: 1.3110x; 1.0424x over previous
"""BestBuddyLoss Trainium2 kernel (8-core data parallel).

Per image: p1 = unfold(x), p2 = unfold(gt),
q = concat(p2, unfold(down2(gt)), unfold(down4(gt)))  -> [3024, 27].
score(i,j) = d(p1_i,q_j) + d(p2_i,q_j) (squared-L2/d, clamped at 0).
With alpha=beta=1 the clamp/scale don't change the argmin:
  argmin_j score == argmax_j ( <p1_i + p2_i, q_j> - |q_j|^2 )
so each (i,j) tile is one K=33 f32r matmul with lhsT=[(p1+p2)^T; ones],
rhs=[q^T; -|q|^2].  The argmax over j is a single-pass fused custom DVE
op (running-max scan + select(Idx) + accum MAX; last tie wins).

v2 layout: the whole kernel is a software pipeline around the DVE argmax
(the only engine that must stream every score).  Image b+1's prep
(downsample/unfold/squares/bias) is emitted interleaved with image b's
main loop so it runs on Pool/ACT/PE slack; tails are split in tile
ranges so the gather/L1 work overlaps the next image's main loop.  The
unfold shuffles write f32r staging tiles that DMA straight into rr
(no rh intermediate); prep DMAs are batched (2 const + ~10 per image).
"""

import sys

sys.path.insert(0, "/opt/trn_rl_repo")

import numpy as np

import concourse.bacc as bacc
import concourse.mybir as mybir
import concourse.tile as tile
from concourse.bass_utils import run_bass_kernel_spmd

# ---------------- problem constants (hardcoded) ----------------
B_FULL = 16
NCORES = 8
B_LOC = B_FULL // NCORES       # images per core
C, H, W = 3, 144, 144
G = 48                         # patch grid (144/3)
NI = G * G                     # 2304 query patches
D = 27                         # C*3*3
NQ = NI + (G // 2) ** 2 + (G // 4) ** 2  # 3024
KD = 33                        # contraction: 27 data + 5 zero + bias@32
KZ = 32                        # bias row partition (32-aligned)
JT = 504                       # j tile; 6 per row
NJT = NQ // JT                 # 6
IT = 128
NIT = NI // IT                 # 18
CUBIC_W = np.array([-0.09375, 0.59375, 0.59375, -0.09375], dtype=np.float32)

F32 = mybir.dt.float32
F32R = mybir.dt.float32r
I16 = mybir.dt.int16
ADD = mybir.AluOpType.add
SUB = mybir.AluOpType.subtract
MUL = mybir.AluOpType.mult
ABS = mybir.ActivationFunctionType.Abs
SQ = mybir.ActivationFunctionType.Square

# packA column layout
_CA_CD2, _CA_CD4, _CA_IDN, _CA_PMT, _CA_ONE = 0, 72, 108, 236, 1260
CA_COLS = 1261

# ---------------- custom DVE op: single-pass argmax -------------------
from concourse.dve_spec import Spec, Src0, Idx, MaxNeg, select, scan, AluOp, maxx, lower
from concourse.dve_uop import DveOpSpec
import concourse.dve_ops as dve_ops
from concourse.dve_ops import DveOp


def _argmax_ref(in0, in1, c0, c1, c2):
    run = np.maximum.accumulate(in0, axis=-1)
    out = np.where(in0 >= run, np.arange(in0.shape[-1], dtype=np.float32),
                   -np.finfo(np.float32).max)
    acc = out.reshape(out.shape[0], -1).max(axis=-1, keepdims=True)
    return out.astype(np.float32), acc.astype(np.float32)


def _register_argmax_op():
    name = "ANT_ARGMAX_LAST"
    if name in dve_ops._SUB_OPCODE_FOR_NAME:
        return next(op for op in dve_ops.OPS if op.name == name)
    body = select(Src0 >= scan(AluOp.MAX, Src0), Idx, MaxNeg)
    spec = Spec(body=body, accum=maxx, reference=_argmax_ref)
    opcode = dve_ops._CUSTOM_DVE_ROW_BASE + len(dve_ops.OPS)
    shas = {v: DveOpSpec(name=name, opcode=opcode, uops=lower(spec, ver=v),
                         rd1_en=False).sha(v) for v in ("v3", "v4")}
    op = DveOp(name, spec, subdim=False, uops_sha=shas)
    dve_ops.OPS.append(op)
    dve_ops._SUB_OPCODE_FOR_NAME[name] = opcode
    dve_ops.CUSTOM_DVE_SPECS[name] = spec
    return op


ARGMAX_OP = _register_argmax_op()

# ---------------- host-side constants ---------------------------------


def _down_matrix(n, f):
    """M[h, i]: out[i] = sum_h M[h, i] * in[h]  (torch bicubic, offset t=.5)."""
    out_n = n // f
    M = np.zeros((n, out_n), dtype=np.float32)
    for i in range(out_n):
        base = f * i + (f // 2 - 1)
        for a in range(4):
            h = min(max(base + a - 1, 0), n - 1)
            M[h, i] += CUBIC_W[a]
    return M


def _perm_matrices():
    """PMT[:, m*128 + r]: one-hot at row (m*16 + r%16) -> out_m = Pm @ v."""
    P = np.zeros((128, 8 * 128), dtype=np.float32)
    for m in range(8):
        for r in range(128):
            P[m * 16 + r % 16, m * 128 + r] = 1.0
    return P


def make_consts():
    cd2 = _down_matrix(H, 2)   # [144, 72]
    cd4 = _down_matrix(H, 4)   # [144, 36]
    packA = np.zeros((128, CA_COLS), dtype=np.float32)
    packA[:, _CA_CD2:_CA_CD2 + 72] = cd2[0:128]
    packA[:, _CA_CD4:_CA_CD4 + 36] = cd4[0:128]
    packA[:, _CA_IDN:_CA_IDN + 128] = np.eye(128, dtype=np.float32)
    packA[:, _CA_PMT:_CA_PMT + 1024] = _perm_matrices()
    packA[:, _CA_ONE] = 1.0
    packB = np.zeros((16, 108), dtype=np.float32)
    packB[:, 0:72] = cd2[128:144]
    packB[:, 72:108] = cd4[128:144]
    padrows = np.zeros((6, NQ), dtype=np.float32)
    padrows[5, :] = 1.0
    return {"packA": np.ascontiguousarray(packA),
            "packB": np.ascontiguousarray(packB),
            "padrows": padrows}


# ---------------- kernel construction ---------------------------------


def build_nc(debug=False):
    nc = bacc.Bacc("TRN2", target_bir_lowering=False)

    x_d = nc.dram_tensor("x", [B_LOC, C, H, W], F32, kind="ExternalInput")
    gt_d = nc.dram_tensor("gt", [B_LOC, C, H, W], F32, kind="ExternalInput")
    pa_d = nc.dram_tensor("packA", [128, CA_COLS], F32, kind="ExternalInput")
    pb_d = nc.dram_tensor("packB", [16, 108], F32, kind="ExternalInput")
    pr_d = nc.dram_tensor("padrows", [6, NQ], F32, kind="ExternalInput")
    d2_d = nc.dram_tensor("scr_d2", [C, 72, 72], F32, kind="Internal")
    d4_d = nc.dram_tensor("scr_d4", [C, 36, 36], F32, kind="Internal")
    p1_d = nc.dram_tensor("scr_p1", [B_LOC, D, NI], F32R, kind="Internal")
    loss_d = nc.dram_tensor("loss", [1, 1], F32, kind="ExternalOutput")
    dbg = {}
    if debug:
        dbg["idxf"] = nc.dram_tensor("dbg_idxf", [B_LOC, 128, NIT], F32,
                                     kind="ExternalOutput")
        dbg["rr"] = nc.dram_tensor("dbg_rr", [B_LOC, KD, NQ], F32,
                                   kind="ExternalOutput")
        dbg["p1t"] = nc.dram_tensor("dbg_p1t", [B_LOC, D, NI], F32,
                                    kind="ExternalOutput")
        dbg["lr"] = nc.dram_tensor("dbg_lr", [B_LOC, KD, NI], F32,
                                   kind="ExternalOutput")
        dbg["part"] = nc.dram_tensor("dbg_part", [D, 2 * B_LOC], F32,
                                     kind="ExternalOutput")

    with tile.TileContext(nc) as tc:
        with (
            tc.tile_pool(name="consts", bufs=1) as cpool,
            tc.tile_pool(name="persist", bufs=1) as ppool,   # per-image slots
            tc.tile_pool(name="stage", bufs=1) as stpool,    # unfold staging
            tc.tile_pool(name="dsw", bufs=1) as dpool,       # downsample work
            tc.tile_pool(name="score", bufs=2) as scpool,
            tc.tile_pool(name="small", bufs=2) as smpool,
            tc.tile_pool(name="psmain", bufs=2, space="PSUM") as psm,
            tc.tile_pool(name="pssmall", bufs=2, space="PSUM") as pss,
        ):
            # ---- consts ----
            pa = cpool.tile([128, CA_COLS], F32, tag="pa")
            pb = cpool.tile([16, 108], F32, tag="pb")
            nc.scalar.dma_start(pa[:], pa_d[:])
            nc.scalar.dma_start(pb[:], pb_d[:])
            cd2a = pa[:, _CA_CD2:_CA_CD2 + 72]
            cd4a = pa[:, _CA_CD4:_CA_CD4 + 36]
            idn = pa[:, _CA_IDN:_CA_IDN + 128]
            pmt = pa[:, _CA_PMT:_CA_PMT + 1024]
            ones = pa[:, _CA_ONE:_CA_ONE + 1]
            cd2b = pb[:, 0:72]
            cd4b = pb[:, 72:108]

            ngo = cpool.tile([D, 1], F32R, tag="ngo")
            nc.gpsimd.memset(ngo[:].bitcast(F32), -1.0)

            junk = cpool.tile([128, NQ], F32, tag="junk")
            junkd = cpool.tile([D, IT * 9], F32, tag="junkd")
            part = cpool.tile([D, 5], F32, tag="part")

            # ---- per-image persistent slots ----
            rr = [ppool.tile([KD, NQ], F32R, tag=f"rr{b}", name=f"rr{b}")
                  for b in range(B_LOC)]
            lr = [ppool.tile([KD, NI], F32R, tag=f"lr{b}", name=f"lr{b}")
                  for b in range(B_LOC)]
            qsq_t = ppool.tile([D, NQ], F32R, tag="qsq", name="qsq")
            qsq = [qsq_t, qsq_t]  # shared; image1 write WARs image0's reads
            idxf = [ppool.tile([128, NIT], F32, tag=f"idxf{b}", name=f"idxf{b}")
                    for b in range(B_LOC)]
            # pad rows (rr 27..32 zero; lr 27..32 zero + bias row 32 = +1)
            # filled once per slot via SWDGE casting DMAs from a DRAM const:
            # zero engine time, and no WAW against the row 0..27 writers.
            for b in range(B_LOC):
                nc.gpsimd.dma_start(rr[b][D:KZ, :], pr_d[0:5, :])
                nc.gpsimd.dma_start(lr[b][D:KD, :], pr_d[0:6, 0:NI])

            # ---- unfold staging tiles: two independent pairs (gt / x
            # paths); d2/d4 unfolds reuse slices of them ----
            at = [stpool.tile([9, 3456], F32, tag=f"at{h}", name=f"at{h}")
                  for h in range(2)]
            bt = [stpool.tile([9, 3456], F32R, tag=f"bt{h}", name=f"bt{h}")
                  for h in range(2)]
            dstg = stpool.tile([9, 1728], F32, tag="dstg")
            dbt = stpool.tile([9, 1728], F32R, tag="dbt")

            # ---- downsample work tiles ----
            ga = dpool.tile([128, C * W], F32, tag="ga")
            gb = dpool.tile([16, C * W], F32, tag="gb")
            gh = dpool.tile([72, C * W], F32, tag="gh")
            ghta = dpool.tile([128, C * 72], F32, tag="ghta")
            ghtb = dpool.tile([16, C * 72], F32, tag="ghtb")
            g2 = dpool.tile([72, C * 72], F32, tag="g2")     # d2 image
            g4 = dpool.tile([36, C * 36], F32, tag="g4")     # d4 image

            def downsample(f, dst, E):
                """ga/gb [128/16, C*W] SBUF --bicubic/f--> dst [n, C*n]."""
                n = H // f
                cda = cd2a if f == 2 else cd4a
                cdb = cd2b if f == 2 else cd4b
                ghp = pss.tile([128, 512], F32, tag="ps", name="ghp")
                nc.tensor.matmul(ghp[0:n, 0:C * W], cda[:, 0:n], ga[:],
                                 start=True, stop=False)
                nc.tensor.matmul(ghp[0:n, 0:C * W], cdb[:, 0:n], gb[:],
                                 start=False, stop=True)
                E.copy(gh[0:n, :], ghp[0:n, 0:C * W])
                gh3 = gh[:].rearrange("i (c w) -> i c w", c=C)
                tpa = pss.tile([128, 512], F32, tag="ps", name="tpa")
                tpb = pss.tile([128, 512], F32, tag="ps", name="tpb")
                for c in range(C):
                    nc.tensor.transpose(tpa[0:128, c * n:(c + 1) * n],
                                        gh3[0:n, c, 0:128], idn[0:n, 0:n])
                    nc.tensor.transpose(tpb[0:16, c * n:(c + 1) * n],
                                        gh3[0:n, c, 128:144], idn[0:n, 0:n])
                E.copy(ghta[:, 0:C * n], tpa[0:128, 0:C * n])
                E.copy(ghtb[:, 0:C * n], tpb[0:16, 0:C * n])
                ghta3 = ghta[:].rearrange("w (c i) -> w c i", c=C)
                ghtb3 = ghtb[:].rearrange("w (c i) -> w c i", c=C)
                op = pss.tile([128, 512], F32, tag="ps", name="op")
                for c in range(C):
                    nc.tensor.matmul(op[0:n, c * n:(c + 1) * n],
                                     ghta3[:, c, 0:n], cda[:, 0:n],
                                     start=True, stop=False)
                    nc.tensor.matmul(op[0:n, c * n:(c + 1) * n],
                                     ghtb3[:, c, 0:n], cdb[:, 0:n],
                                     start=False, stop=True)
                E.copy(dst[:, :], op[0:n, 0:C * n])

            def unfold_in(src_dram, hf, pair):
                """Queue the 3 stage-in DMAs for one unfold half."""
                a = at[pair]
                src = src_dram.rearrange("c (gi r) w -> c r gi w", r=3)
                for c in range(C):
                    nc.sync.dma_start(
                        a[3 * c:3 * c + 3, :],
                        src[c, :, hf * 24:(hf + 1) * 24, :])

            def unfold_out(dst_ap, E, pair, spill=None):
                """Shuffle the staged half and DMA it out to dst [27, 1152].
                spill: optional extra DRAM destination (exact p1 copy)."""
                a, o = at[pair], bt[pair]
                av = a[:].rearrange("p (gi gj s) -> p s gi gj", gi=24, gj=48)
                ov = o[:].rearrange("p (s gi gj) -> p s gi gj", s=3, gi=24)
                if E is nc.scalar:
                    E.copy(ov, av)
                else:
                    E.tensor_copy(ov, av)
                nc.sync.dma_start(dst_ap, o[:])
                if spill is not None:
                    nc.sync.dma_start(spill, o[:])

            def unfold_small(img, n, dst_ap, E):
                """Unfold of img [n, C*n] (n=72 d2 / 36 d4) via a DRAM bounce
                (the (gi r)->(c r) partition regroup isn't one DMA)."""
                g = n // 3
                sz = g * n                       # per-channel elements
                scr = d2_d if n == 72 else d4_d
                nc.sync.dma_start(scr.rearrange("c i j -> i c j"), img[:])
                for c in range(C):
                    nc.sync.dma_start(
                        dstg[3 * c:3 * c + 3, 0:sz],
                        scr[c].rearrange("(gi r) j -> r gi j", r=3))
                av = dstg[:, 0:sz].rearrange("p (gi gj s) -> p s gi gj",
                                             gi=g, gj=g)
                ov = dbt[:, 0:sz].rearrange("p (s gi gj) -> p s gi gj",
                                            s=3, gi=g)
                E.tensor_copy(ov, av)
                nc.sync.dma_start(dst_ap, dbt[:, 0:sz])

            def squares(b, lo, hi, E):
                if E is nc.scalar:
                    nc.scalar.activation(qsq[b][:, lo:hi],
                                         rr[b][0:D, lo:hi].bitcast(F32), SQ)
                else:
                    E.tensor_tensor(qsq[b][:, lo:hi],
                                    rr[b][0:D, lo:hi].bitcast(F32),
                                    rr[b][0:D, lo:hi].bitcast(F32), op=MUL)

            def bias_chunk(b, jt, E=None):
                bnp = pss.tile([128, 512], F32, tag="ps", name="bnp")
                nc.tensor.matmul(bnp[0:1, 0:JT], ngo[:],
                                 qsq[b][:, jt * JT:(jt + 1) * JT])
                dst = rr[b][KZ:KZ + 1, jt * JT:(jt + 1) * JT]
                if E is nc.vector:
                    E.tensor_copy(dst, bnp[0:1, 0:JT])
                else:
                    nc.scalar.copy(dst, bnp[0:1, 0:JT])

            def load_ds_src(b, issuer=None):
                issuer = issuer or nc.sync
                gsrc = gt_d[b].rearrange("c h w -> h c w")
                issuer.dma_start(ga[:].rearrange("h (c w) -> h c w", c=C),
                                 gsrc[0:128])
                issuer.dma_start(gb[:].rearrange("h (c w) -> h c w", c=C),
                                 gsrc[128:144])

            def lr_fill(b, E):
                # lr rows 0:27 hold p1 (from the x unfold); add p2 in place.
                E.tensor_tensor(lr[b][0:D, :], lr[b][0:D, :].bitcast(F32),
                                rr[b][0:D, 0:NI].bitcast(F32), op=ADD)

            def main_tile(b, t):
                sc = scpool.tile([128, NQ], F32, tag="sc")
                for third in range(2):
                    ps = psm.tile([128, 3, 512], F32, tag="psmain")
                    for k in range(3):
                        jt = third * 3 + k
                        nc.tensor.matmul(
                            ps[:, k, 0:JT],
                            lr[b][:, t * IT:(t + 1) * IT],
                            rr[b][:, jt * JT:(jt + 1) * JT],
                        )
                    nc.scalar.copy(
                        sc[:, third * 3 * JT:(third + 1) * 3 * JT],
                        ps[:, :, 0:JT],
                    )
                nc.vector._custom_dve(
                    ARGMAX_OP, out=junk[:], in0=sc[:],
                    accum_out=idxf[b][:, t:t + 1],
                )

            def tail_range(b, tlo, thi, col, df_eng):
                """Gather + L1 for i-tiles [tlo, thi) -> part[:, col]."""
                nt = thi - tlo
                wp = pss.tile([128, 512], F32, tag="ps", name="wp")
                wp3 = wp[0:128, 0:8 * nt].rearrange("p (m t) -> p m t", m=8)
                for m in range(8):
                    nc.tensor.matmul(wp3[:, m, :],
                                     pmt[:, m * 128:(m + 1) * 128],
                                     idxf[b][:, tlo:thi])
                widx = smpool.tile([128, 8 * 9], I16, tag="widx")
                w3 = widx[:, 0:8 * nt].rearrange("p (t m) -> p t m", t=nt)
                nc.vector.tensor_copy(w3[:, :, :],
                                      wp3[:, :, :].rearrange("p m t -> p t m"))
                sel = smpool.tile([32, IT * 9], F32, tag="sel", bufs=1)
                ni = IT * nt
                nc.gpsimd.ap_gather(
                    sel[:, 0:ni].rearrange("p (n d) -> p n d", d=1),
                    rr[b][0:32, :].bitcast(F32).rearrange(
                        "p (n d) -> p n d", d=1),
                    widx[0:32, 0:8 * nt],
                    channels=32, num_elems=NQ, d=1, num_idxs=ni,
                )
                # exact p1 slice reloaded from the DRAM spill (lr - p2 would
                # amplify the f32r write rounding into the loss)
                df = smpool.tile([D, IT * 9], F32R, tag="df", bufs=1)
                cols = slice(tlo * IT, thi * IT)
                nc.sync.dma_start(df[:, 0:ni], p1_d[b][:, cols])
                df_eng.tensor_tensor(df[:, 0:ni].bitcast(F32),
                                     sel[0:D, 0:ni],
                                     df[:, 0:ni].bitcast(F32), op=SUB)
                nc.scalar.activation(junkd[:, 0:ni],
                                     df[:, 0:ni].bitcast(F32), ABS,
                                     accum_out=part[0:D, col:col + 1])

            # ================= schedule =================
            # ---- prep image 0 (exposed; spread across engines).  All six
            # stage-in DMA groups are queued before any dependent out-DMA so
            # the in-order SP queue never head-blocks a load behind a
            # compute-dependent store. ----
            load_ds_src(0, nc.scalar)   # ACT queue: dep-free early loads
            downsample(4, g4, nc.scalar)
            # gt halves stage through BOTH pairs so their loads don't
            # serialize behind each other's shuffles; x reuses them after.
            unfold_in(gt_d[0], 0, 0)
            unfold_in(gt_d[0], 1, 1)
            unfold_out(rr[0][0:D, 0:1152], nc.vector, 0)
            unfold_in(x_d[0], 0, 0)
            unfold_out(rr[0][0:D, 1152:NI], nc.vector, 1)
            unfold_in(x_d[0], 1, 1)
            unfold_out(lr[0][0:D, 0:1152], nc.vector, 0,
                       p1_d[0][:, 0:1152])
            unfold_out(lr[0][0:D, 1152:NI], nc.vector, 1,
                       p1_d[0][:, 1152:NI])
            downsample(2, g2, nc.scalar)
            squares(0, 0, NI, nc.scalar)
            for jt in range(4):
                bias_chunk(0, jt)
            unfold_small(g2, 72, rr[0][0:D, NI:NI + 576], nc.gpsimd)
            unfold_small(g4, 36, rr[0][0:D, NI + 576:NQ], nc.gpsimd)
            lr_fill(0, nc.vector)
            squares(0, NI, NI + 576, nc.scalar)   # d2 part
            squares(0, NI + 576, NQ, nc.scalar)   # d4 part
            for jt in range(4, NJT):
                bias_chunk(0, jt)

            from contextlib import contextmanager

            @contextmanager
            def low_prio(off=1 << 20):
                # deprioritized vs the main mm->copy->argmax chain: runs only
                # in engine slack, never wins a tie against main-loop work.
                old = tc.cur_priority
                tc.cur_priority = old + off
                try:
                    yield
                finally:
                    tc.cur_priority = old

            # ---- main(0) with prep(1) interleaved (image-1 prep runs on
            # Pool shuffles + ACT copy slack + PE slack) ----
            for t in range(NIT):
                main_tile(0, t)
                if t == 1:
                    with low_prio():
                        unfold_in(gt_d[1], 0, 0)
                        unfold_in(x_d[1], 0, 1)
                elif t == 2:
                    with low_prio():
                        load_ds_src(1)
                        unfold_out(rr[1][0:D, 0:1152], nc.gpsimd, 0)
                        unfold_in(gt_d[1], 1, 0)
                        unfold_out(lr[1][0:D, 0:1152], nc.gpsimd, 1,
                                   p1_d[1][:, 0:1152])
                        unfold_in(x_d[1], 1, 1)
                elif t == 3:
                    with low_prio():
                        downsample(2, g2, nc.scalar)
                        unfold_out(rr[1][0:D, 1152:NI], nc.gpsimd, 0)
                        unfold_out(lr[1][0:D, 1152:NI], nc.gpsimd, 1,
                                   p1_d[1][:, 1152:NI])
                elif t == 4:
                    with low_prio():
                        downsample(4, g4, nc.scalar)
                        squares(1, 0, NI, nc.gpsimd)
                elif t == 5:
                    with low_prio():
                        unfold_small(g2, 72, rr[1][0:D, NI:NI + 576],
                                     nc.gpsimd)
                        unfold_small(g4, 36, rr[1][0:D, NI + 576:NQ],
                                     nc.gpsimd)
                elif t == 6:
                    with low_prio():
                        lr_fill(1, nc.gpsimd)
                        squares(1, NI, NQ, nc.gpsimd)
                elif t == 7:
                    with low_prio():
                        for jt in range(NJT):
                            bias_chunk(1, jt)
                elif t == 11:
                    with low_prio():
                        tail_range(0, 0, 9, 0, nc.gpsimd)

            # ---- main(1); image-0 tail B then image-1 tails ----
            for t in range(NIT):
                main_tile(1, t)
                if t == 0:
                    with low_prio():
                        tail_range(0, 9, NIT, 1, nc.gpsimd)
                elif t == 11:
                    with low_prio():
                        tail_range(1, 0, 9, 2, nc.gpsimd)
                elif t == 15:
                    with low_prio():
                        tail_range(1, 9, 15, 3, nc.gpsimd)
            tail_range(1, 15, NIT, 4, nc.vector)

            # ---- final loss ----
            fin = pss.tile([128, 512], F32, tag="ps", name="fin")
            nc.tensor.matmul(fin[0:1, 0:5], ones[0:D, :], part[0:D, 0:5])
            lt = smpool.tile([1, 1], F32, tag="loss")
            ltj = smpool.tile([1, 5], F32, tag="ltj")
            nc.scalar.activation(ltj[:], fin[0:1, 0:5], ABS,
                                 accum_out=lt[:])
            nc.sync.dma_start(loss_d[:], lt[:])

            if debug:
                for b in range(B_LOC):
                    nc.sync.dma_start(dbg["idxf"][b], idxf[b][:, :])
                    nc.sync.dma_start(dbg["rr"][b], rr[b][:, :].bitcast(F32))
                    nc.sync.dma_start(dbg["lr"][b], lr[b][:, :].bitcast(F32))
                nc.sync.dma_start(dbg["part"][:, :], part[:, :])

    nc.compile()
    return nc


_NC_CACHE = None


def _get_nc():
    global _NC_CACHE
    if _NC_CACHE is None:
        _NC_CACHE = build_nc()
    return _NC_CACHE


# part column layout: (b, range) -> col
PART_COLS = 5  # [b0 t0:9, b0 t9:18, b1 t0:9, b1 t9:15, b1 t15:18]


def kernel(x: np.ndarray, gt: np.ndarray, _trace=False, _debug=False):
    x = np.ascontiguousarray(np.asarray(x, dtype=np.float32))
    gt = np.ascontiguousarray(np.asarray(gt, dtype=np.float32))
    consts = make_consts()
    nc = build_nc(debug=True) if _debug else _get_nc()
    in_maps = []
    for c in range(NCORES):
        m = {"x": x[c * B_LOC:(c + 1) * B_LOC],
             "gt": gt[c * B_LOC:(c + 1) * B_LOC]}
        m.update(consts)
        in_maps.append(m)
    res = run_bass_kernel_spmd(
        nc, in_maps, core_ids=list(range(NCORES)), trace=_trace,
        trace_cores=[0] if _trace else None,
    )
    total = sum(float(r["loss"][0, 0]) for r in res.results)
    out = np.asarray(np.float32(total / (B_FULL * NI * D)))
    if _debug or _trace:
        return out, res
    return out


if __name__ == "__main__":
    xs = np.load("/root/problem/work/x.npy")
    gts = np.load("/root/problem/work/gt.npy")
    expected = float(np.load("/root/problem/work/expected.npy"))
    got = float(kernel(xs, gts))
    rel = abs(got - expected) / abs(expected)
    print(f"expected {expected:.8f}  got {got:.8f}  relerr {rel:.3e}")
